# revision 1
# baseline (speedup 1.0000x reference)
"""BiLSTM-CRF forward loss on 8 Trainium2 cores (batch-parallel SPMD).

Layout/sharding summary (per core, b=8 examples of B=64):
- embedding gather -> x^T (PE transposes) -> L1 input-projection GEMM (bf16)
- L1 BiLSTM scan: col-tiled state-stationary matmuls (4 strips of 8
  partitions), gates layout [128p=(4 strips x 8b), 512f=(i|f|o|g)*128]
- L2 BiLSTM scan: both directions packed in one 128-partition tile
- linear -> logits^T [48, T*8] and logits [T*8, 48]
- CRF forward pass in exp-domain: alpha_t = (expT^T @ alpha) * exp(emit_t),
  one bf16 matmul + one DVE mul per step; logZ via log of the final sum
- gold path score via indirect-DMA gathers + selector matmuls
Outputs per core: [2, 8] fp32 (row0 joint, row1 logZ). Host sums
-(joint - logZ) over all 64 examples.
"""

import numpy as np

B, T, VOCAB, EMBED, HID, TAGS = 64, 512, 30000, 512, 1024, 48
H1, H2 = HID // 2, HID // 4  # 512, 256
BPC = B // 8  # batch per core = 8
NTOK = T * BPC  # 4096 tokens per core
LN48 = float(np.log(48.0))

_CACHE = {}


def _gate_perm(h):
    """Permutation p such that W[p] has strip layout:
    strip j (512 cols) = [i_j | f_j | o_j | g_j], each 128 units of gate
    blocks taken from pytorch (i,f,g,o) row order. h = per-dir hidden."""
    nj = (4 * h) // 512
    slots = [0, 1, 3, 2]  # i, f, o, g source gate index
    p = []
    for j in range(nj):
        for g_idx in slots:
            base = g_idx * h + j * 128
            p.extend(range(base, base + 128))
    return np.array(p, dtype=np.int64)


def _build_program():
    import concourse.bass as bass
    import concourse.tile as tile
    import concourse.mybir as mybir
    from concourse.vector_clock import ScopedClock, VectorClock
    from concourse.masks import make_identity

    def _patched_drain_and_barrier(self, tick_clock, wait_clock):
        # This container's walrus rejects >2 sem waits on one CTRL
        # instruction; split the kernel-tail drain waits into per-proc
        # NOP waits on the same (in-order) SP queue.
        vc = tick_clock.global_clock
        n = len(vc)
        for p in range(n):
            t = vc[p]
            if t > 0:
                vec = [0] * n
                vec[p] = t
                nop = self.nc.sync.nop()
                wait_clock.add_sem_waits(nop.ins, ScopedClock({None: VectorClock(vec)}))
        self.nc.sync.drain()
        self.nc.all_engine_barrier()
        popped = self.nc._tile_sem_poison_stack.pop()
        assert popped is self._sem_poison
        self.nc.clear_and_free_semaphores(list(self.sems.allocated().values()))
        self.nc.all_engine_barrier()

    tile.TileContext._drain_and_barrier = _patched_drain_and_barrier

    f32 = mybir.dt.float32
    bf16 = mybir.dt.bfloat16
    i32 = mybir.dt.int32
    ACT = mybir.ActivationFunctionType
    ADD = mybir.AluOpType.add
    MULT = mybir.AluOpType.mult

    nc = bass.Bass()
    PH = int(__import__("os").environ.get("KPHASES", "99"))

    # ---------------- inputs ----------------
    def din(name, shape, dt=f32):
        return nc.dram_tensor(name, shape, dt, kind="ExternalInput")

    embed_bf = din("embed_bf", [VOCAB, EMBED], bf16)
    seq_tok = din("seq_tok", [NTOK, 1], i32)
    idx_emit = din("idx_emit", [NTOK, 1], i32)
    idx_trans = din("idx_trans", [NTOK, 1], i32)
    idx_start = din("idx_start", [BPC, 1], i32)
    idx_end = din("idx_end", [BPC, 1], i32)
    wihT1 = din("wihT1", [2, EMBED, 4 * H1], bf16)
    whhT1 = din("whhT1", [2, H1, 4 * H1], bf16)
    bias1 = din("bias1", [2, 1, 4 * H1], bf16)
    wihT2 = din("wihT2", [2, HID, 4 * H2], bf16)
    whhT2 = din("whhT2", [2, H2, 4 * H2], bf16)
    bias2 = din("bias2", [2, 1, 4 * H2], bf16)
    linWT = din("linWT", [H2 * 2, TAGS], bf16)
    lin_b = din("lin_b", [1, TAGS], bf16)
    c0_l1 = din("c0_l1", [2, 128, 128])  # strip-packed
    h0T_l1 = din("h0T_l1", [2, H1, BPC], bf16)
    c0_l2 = din("c0_l2", [128, 128])  # both dirs strip-packed
    h0T_l2 = din("h0T_l2", [2, H2, BPC], bf16)
    trans_d = din("transitions", [TAGS, TAGS])
    start_d = din("start_trans", [TAGS, 1])
    end_d = din("end_trans", [TAGS, 1])
    sel_d = din("sel", [128, BPC])  # sel[p, b] = (p % 8 == b), fp32
    mask_d = din("mask_last", [128, 1])  # 1.0 except rows 120..127 -> 0.0

    out_d = nc.dram_tensor("out", [2, BPC], f32, kind="ExternalOutput")

    NM = NTOK // 128  # 32 token chunks

    with tile.TileContext(nc) as tc:
        with tc.tile_pool(name="dram", bufs=1, space="DRAM") as dpool, \
             tc.tile_pool(name="const", bufs=1) as cpool, \
             tc.tile_pool(name="persist", bufs=1) as ppool:

            xT_t = dpool.tile([EMBED, NTOK], bf16)          # 4 MB
            ih1_t = dpool.tile([2, T, 128, 512], bf16)      # 134 MB padded
            ih2_t = dpool.tile([T, 128, 512], bf16)         # 67 MB padded
            logits_t = dpool.tile([NTOK, TAGS], f32)

            ones_bf = cpool.tile([1, 512], bf16)
            nc.gpsimd.memset(ones_bf[:], 1.0)
            ones_f = cpool.tile([128, 1], f32)
            nc.gpsimd.memset(ones_f[:], 1.0)
            id8 = cpool.tile([128, 8], bf16)
            nc.gpsimd.memset(id8[:], 0.0)
            for j in range(4):
                make_identity(nc, id8[32 * j:32 * j + 8, :], nomemset=True)
            id128 = cpool.tile([128, 128], bf16)
            make_identity(nc, id128[:])
            sel_sb = cpool.tile([128, BPC], f32)
            nc.sync.dma_start(sel_sb[:], sel_d[:])
            mask_sb = cpool.tile([128, 1], f32)
            nc.sync.dma_start(mask_sb[:], mask_d[:])

            # h1T / h2T live in SBUF: [128, NTOK] bf16 tiles per 128-unit chunk
            h1T = [ppool.tile([128, NTOK], bf16, tag=f"h1T{i}", name=f"h1T{i}") for i in range(8)]
            h2T = [ppool.tile([128, NTOK], bf16, tag=f"h2T{i}", name=f"h2T{i}") for i in range(4)]

            # ================= P1: embedding gather + transpose =============
            with tc.tile_pool(name="p1", bufs=3) as sp, \
                 tc.tile_pool(name="p1p", bufs=4, space="PSUM") as psp:
                for m in range(NM if PH >= 1 else 0):
                    idx = sp.tile([128, 1], i32, tag="idx")
                    nc.sync.dma_start(idx[:], seq_tok[128 * m:128 * (m + 1), :])
                    xg = sp.tile([128, EMBED], bf16, tag="xg")
                    nc.gpsimd.indirect_dma_start(
                        out=xg[:], out_offset=None, in_=embed_bf[:],
                        in_offset=bass.IndirectOffsetOnAxis(ap=idx[:, :1], axis=0))
                    for e in range(EMBED // 128):
                        pt = psp.tile([128, 128], bf16, space="PSUM", tag="pt")
                        nc.tensor.transpose(out=pt[:], in_=xg[:, 128 * e:128 * (e + 1)],
                                            identity=id128[:])
                        xs = sp.tile([128, 128], bf16, tag="xs")
                        nc.vector.tensor_copy(xs[:], pt[:])
                        nc.sync.dma_start(
                            xT_t[128 * e:128 * (e + 1), 128 * m:128 * (m + 1)], xs[:])

            # ================= P2: L1 input GEMM ===========================
            with tc.tile_pool(name="w2", bufs=1) as wp, \
                 tc.tile_pool(name="p2", bufs=3) as sp, \
                 tc.tile_pool(name="p2p", bufs=4, space="PSUM") as psp:
                for d in range(2 if PH >= 2 else 0):
                    wt = [wp.tile([128, 4 * H1], bf16, tag=f"w1_{d}_{k}", name=f"w1_{d}_{k}") for k in range(4)]
                    for k in range(4):
                        nc.sync.dma_start(wt[k][:], wihT1[d, 128 * k:128 * (k + 1), :])
                    bt = wp.tile([1, 4 * H1], bf16, tag=f"b1_{d}")
                    nc.sync.dma_start(bt[:], bias1[d])
                    for m in range(NM):
                        xs = sp.tile([128, EMBED], bf16, tag="xs")
                        nc.sync.dma_start(
                            xs[:].rearrange("p (k n) -> p k n", k=4),
                            xT_t[:, 128 * m:128 * (m + 1)].rearrange(
                                "(k p) n -> p k n", p=128))
                        gsb = sp.tile([128, 4 * H1], bf16, tag="gsb")
                        for jj in range(4):
                            pg = psp.tile([128, 512], f32, space="PSUM", tag="pg")
                            for k in range(4):
                                nc.tensor.matmul(
                                    pg[:], lhsT=xs[:, 128 * k:128 * (k + 1)],
                                    rhs=wt[k][:, 512 * jj:512 * (jj + 1)],
                                    start=(k == 0), stop=False)
                            nc.tensor.matmul(
                                pg[:], lhsT=ones_bf[0:1, 0:128],
                                rhs=bt[0:1, 512 * jj:512 * (jj + 1)],
                                start=False, stop=True)
                            nc.scalar.copy(gsb[:, 512 * jj:512 * (jj + 1)], pg[:])
                        t0 = m * 16
                        for j in range(4):
                            dst = ih1_t[d, t0:t0 + 16].rearrange(
                                "t (j q) u -> t j q u", j=4)[:, j, 0:BPC, :]
                            nc.sync.dma_start(
                                dst, gsb[:, 512 * j:512 * (j + 1)])

            # ================= P3: L1 scans (fwd + bwd) ====================
            with tc.tile_pool(name="w3", bufs=1) as wp, \
                 tc.tile_pool(name="st3", bufs=1) as stp, \
                 tc.tile_pool(name="p3", bufs=4) as sp, \
                 tc.tile_pool(name="p3g", bufs=2, space="PSUM") as psg, \
                 tc.tile_pool(name="p3t", bufs=4, space="PSUM") as pst:
                whh = {}
                for d in range(2):
                    for k in range(4):
                        w = wp.tile([128, 4 * H1], bf16, tag=f"whh1_{d}_{k}")
                        nc.sync.dma_start(w[:], whhT1[d, 128 * k:128 * (k + 1), :])
                        whh[(d, k)] = w
                cS = {}
                hT0 = {}
                for d in range(2):
                    for par in range(2):
                        c = stp.tile([128, 128], f32, tag=f"c1_{d}_{par}")
                        cS[(d, par)] = c
                    nc.sync.dma_start(cS[(d, 0)][:], c0_l1[d])
                    h0 = stp.tile([128, 32], bf16, tag=f"h0T1_{d}")
                    for k in range(4):
                        nc.sync.dma_start(h0[:, 8 * k:8 * (k + 1)],
                                          h0T_l1[d, 128 * k:128 * (k + 1), :])
                    hT0[d] = h0

                for s in range(T if PH >= 3 else 0):
                    for d in range(2):
                        t = s if d == 0 else T - 1 - s
                        # lhsT source: previous hidden state transposed
                        if s == 0:
                            lhsT_of = lambda k, d=d: hT0[d][:, 8 * k:8 * (k + 1)]
                        else:
                            tp = (s - 1) if d == 0 else (T - s)
                            lhsT_of = (lambda k, d=d, tp=tp:
                                       h1T[d * 4 + k][:, 8 * tp:8 * tp + 8])
                        ih = sp.tile([128, 512], bf16, tag=f"ih1_{d}")
                        nc.sync.dma_start(ih[:], ih1_t[d, t])
                        pg = psg.tile([128, 512], f32, space="PSUM", tag=f"pg_{d}")
                        for jj in range(4):
                            for k in range(4):
                                nc.tensor.matmul(
                                    pg[32 * jj:32 * jj + 8, :], lhsT=lhsT_of(k),
                                    rhs=whh[(d, k)][:, 512 * jj:512 * (jj + 1)],
                                    start=(k == 0), stop=(k == 3),
                                    tile_position=(0, 32 * jj))
                        gsb = sp.tile([128, 512], bf16, tag=f"g1_{d}")
                        nc.vector.tensor_tensor(out=gsb[:], in0=pg[:], in1=ih[:], op=ADD)
                        sig = sp.tile([128, 384], bf16, tag=f"sig1_{d}")
                        nc.scalar.activation(sig[:], gsb[:, 0:384], ACT.Sigmoid)
                        tg = sp.tile([128, 128], bf16, tag=f"tg1_{d}")
                        nc.scalar.activation(tg[:], gsb[:, 384:512], ACT.Tanh)
                        c_old = cS[(d, s % 2)]
                        c_new = cS[(d, (s + 1) % 2)]
                        t1 = sp.tile([128, 128], f32, tag=f"t1_{d}")
                        nc.vector.tensor_tensor(out=t1[:], in0=sig[:, 128:256], in1=c_old[:], op=MULT)
                        t2 = sp.tile([128, 128], f32, tag=f"t2_{d}")
                        nc.vector.tensor_tensor(out=t2[:], in0=sig[:, 0:128], in1=tg[:], op=MULT)
                        nc.vector.tensor_tensor(out=c_new[:], in0=t1[:], in1=t2[:], op=ADD)
                        th = sp.tile([128, 128], bf16, tag=f"th1_{d}")
                        nc.scalar.activation(th[:], c_new[:], ACT.Tanh)
                        h = sp.tile([128, 128], bf16, tag=f"h1_{d}")
                        nc.vector.tensor_tensor(out=h[:], in0=sig[:, 256:384], in1=th[:], op=MULT)
                        for k in range(4):
                            pt = pst.tile([128, 8], bf16, space="PSUM", tag="pt3")
                            nc.tensor.transpose(out=pt[:], in_=h[32 * k:32 * k + 8, :],
                                                identity=id8[32 * k:32 * k + 8, :],
                                                tile_position=(32 * k, 0))
                            if k % 2 == 0:
                                nc.vector.tensor_copy(h1T[d * 4 + k][:, 8 * t:8 * t + 8], pt[:])
                            else:
                                nc.scalar.copy(h1T[d * 4 + k][:, 8 * t:8 * t + 8], pt[:])

            # ================= P4: L2 input GEMM ===========================
            with tc.tile_pool(name="w4", bufs=1) as wp, \
                 tc.tile_pool(name="p4", bufs=3) as sp, \
                 tc.tile_pool(name="p4p", bufs=4, space="PSUM") as psp:
                for d in range(2 if PH >= 4 else 0):
                    wt = [wp.tile([128, 4 * H2], bf16, tag=f"w2_{d}_{k}", name=f"w2_{d}_{k}") for k in range(8)]
                    for k in range(8):
                        nc.sync.dma_start(wt[k][:], wihT2[d, 128 * k:128 * (k + 1), :])
                    bt = wp.tile([1, 4 * H2], bf16, tag=f"b2_{d}")
                    nc.sync.dma_start(bt[:], bias2[d])
                    for m in range(NM):
                        gsb = sp.tile([128, 4 * H2], bf16, tag="g2sb")
                        for jj in range(2):
                            pg = psp.tile([128, 512], f32, space="PSUM", tag="pg4")
                            for k in range(8):
                                nc.tensor.matmul(
                                    pg[:], lhsT=h1T[k][:, 128 * m:128 * (m + 1)],
                                    rhs=wt[k][:, 512 * jj:512 * (jj + 1)],
                                    start=(k == 0), stop=False)
                            nc.tensor.matmul(
                                pg[:], lhsT=ones_bf[0:1, 0:128],
                                rhs=bt[0:1, 512 * jj:512 * (jj + 1)],
                                start=False, stop=True)
                            nc.scalar.copy(gsb[:, 512 * jj:512 * (jj + 1)], pg[:])
                        t0 = m * 16
                        for j in range(2):
                            dst = ih2_t[t0:t0 + 16].rearrange(
                                "t (j q) u -> t j q u", j=4)[:, 2 * d + j, 0:BPC, :]
                            nc.sync.dma_start(dst, gsb[:, 512 * j:512 * (j + 1)])

            # ================= P5: L2 scans (both dirs packed) =============
            with tc.tile_pool(name="w5", bufs=1) as wp, \
                 tc.tile_pool(name="st5", bufs=1) as stp, \
                 tc.tile_pool(name="p5", bufs=4) as sp, \
                 tc.tile_pool(name="p5g", bufs=2, space="PSUM") as psg, \
                 tc.tile_pool(name="p5t", bufs=4, space="PSUM") as pst:
                whh2 = {}
                for d in range(2):
                    for k in range(2):
                        w = wp.tile([128, 4 * H2], bf16, tag=f"whh2_{d}_{k}")
                        nc.sync.dma_start(w[:], whhT2[d, 128 * k:128 * (k + 1), :])
                        whh2[(d, k)] = w
                c2 = [stp.tile([128, 128], f32, tag=f"c2_{p}", name=f"c2_{p}") for p in range(2)]
                nc.sync.dma_start(c2[0][:], c0_l2[:])
                h0_2 = stp.tile([128, 32], bf16, tag="h0T2")
                for d in range(2):
                    for k in range(2):
                        nc.sync.dma_start(h0_2[:, 8 * (2 * d + k):8 * (2 * d + k) + 8],
                                          h0T_l2[d, 128 * k:128 * (k + 1), :])

                for s in range(T if PH >= 5 else 0):
                    ih = sp.tile([128, 512], bf16, tag="ih2")
                    tf, tb = s, T - 1 - s
                    nc.sync.dma_start(ih[0:64, :], ih2_t[tf, 0:64, :])
                    nc.sync.dma_start(ih[64:128, :], ih2_t[tb, 64:128, :])
                    pg = psg.tile([128, 512], f32, space="PSUM", tag="pg5")
                    for d in range(2):
                        t = tf if d == 0 else tb
                        for j in range(2):
                            for k in range(2):
                                if s == 0:
                                    lh = h0_2[:, 8 * (2 * d + k):8 * (2 * d + k) + 8]
                                else:
                                    tp = (s - 1) if d == 0 else (T - s)
                                    lh = h2T[2 * d + k][:, 8 * tp:8 * tp + 8]
                                nc.tensor.matmul(
                                    pg[32 * (2 * d + j):32 * (2 * d + j) + 8, :],
                                    lhsT=lh,
                                    rhs=whh2[(d, k)][:, 512 * j:512 * (j + 1)],
                                    start=(k == 0), stop=(k == 1),
                                    tile_position=(0, 32 * (2 * d + j)))
                    gsb = sp.tile([128, 512], bf16, tag="g5")
                    nc.vector.tensor_tensor(out=gsb[:], in0=pg[:], in1=ih[:], op=ADD)
                    sig = sp.tile([128, 384], bf16, tag="sig5")
                    nc.scalar.activation(sig[:], gsb[:, 0:384], ACT.Sigmoid)
                    tg = sp.tile([128, 128], bf16, tag="tg5")
                    nc.scalar.activation(tg[:], gsb[:, 384:512], ACT.Tanh)
                    c_old, c_new = c2[s % 2], c2[(s + 1) % 2]
                    t1 = sp.tile([128, 128], f32, tag="t15")
                    nc.vector.tensor_tensor(out=t1[:], in0=sig[:, 128:256], in1=c_old[:], op=MULT)
                    t2 = sp.tile([128, 128], f32, tag="t25")
                    nc.vector.tensor_tensor(out=t2[:], in0=sig[:, 0:128], in1=tg[:], op=MULT)
                    nc.vector.tensor_tensor(out=c_new[:], in0=t1[:], in1=t2[:], op=ADD)
                    th = sp.tile([128, 128], bf16, tag="th5")
                    nc.scalar.activation(th[:], c_new[:], ACT.Tanh)
                    h = sp.tile([128, 128], bf16, tag="h5")
                    nc.vector.tensor_tensor(out=h[:], in0=sig[:, 256:384], in1=th[:], op=MULT)
                    for q in range(4):  # q = 2*d + k
                        d, k = q // 2, q % 2
                        t = tf if d == 0 else tb
                        pt = pst.tile([128, 8], bf16, space="PSUM", tag="pt5")
                        nc.tensor.transpose(out=pt[:], in_=h[32 * q:32 * q + 8, :],
                                            identity=id8[32 * q:32 * q + 8, :],
                                            tile_position=(32 * q, 0))
                        if q % 2 == 0:
                            nc.vector.tensor_copy(h2T[q][:, 8 * t:8 * t + 8], pt[:])
                        else:
                            nc.scalar.copy(h2T[q][:, 8 * t:8 * t + 8], pt[:])

            # ================= P6: linear -> logitsT + logits ==============
            logitsT = ppool.tile([TAGS, NTOK], f32, tag="logitsT")
            Esb = ppool.tile([TAGS, NTOK], bf16, tag="Esb")
            with tc.tile_pool(name="w6", bufs=1) as wp, \
                 tc.tile_pool(name="p6", bufs=3) as sp, \
                 tc.tile_pool(name="p6p", bufs=4, space="PSUM") as psp:
                lw = [wp.tile([128, TAGS], bf16, tag=f"lw{k}", name=f"lw{k}") for k in range(4)]
                for k in range(4):
                    nc.sync.dma_start(lw[k][:], linWT[128 * k:128 * (k + 1), :])
                lb = wp.tile([1, TAGS], bf16, tag="lb")
                nc.sync.dma_start(lb[:], lin_b[:])
                # logitsT [48, NTOK]
                for n in range(NTOK // 512 if PH >= 6 else 0):
                    pg = psp.tile([TAGS, 512], f32, space="PSUM", tag="pl")
                    for k in range(4):
                        nc.tensor.matmul(pg[:], lhsT=lw[k][:],
                                         rhs=h2T[k][:, 512 * n:512 * (n + 1)],
                                         start=(k == 0), stop=False)
                    nc.tensor.matmul(pg[:], lhsT=lb[0:1, :], rhs=ones_bf[0:1, :],
                                     start=False, stop=True)
                    nc.scalar.copy(logitsT[:, 512 * n:512 * (n + 1)], pg[:])
                    nc.scalar.activation(Esb[:, 512 * n:512 * (n + 1)],
                                         pg[:], ACT.Exp)
                # logits [NTOK, 48] to DRAM for gathers
                for m in range(NM if PH >= 6 else 0):
                    pg = psp.tile([128, TAGS], f32, space="PSUM", tag="pl2")
                    for k in range(4):
                        nc.tensor.matmul(pg[:], lhsT=h2T[k][:, 128 * m:128 * (m + 1)],
                                         rhs=lw[k][:], start=(k == 0), stop=False)
                    nc.tensor.matmul(pg[:], lhsT=ones_bf[0:1, 0:128], rhs=lb[0:1, :],
                                     start=False, stop=True)
                    ls = sp.tile([128, TAGS], f32, tag="ls")
                    nc.scalar.copy(ls[:], pg[:])
                    nc.sync.dma_start(logits_t[128 * m:128 * (m + 1), :], ls[:])

            # ================= P7: CRF partition (exp domain) ==============
            with tc.tile_pool(name="p7", bufs=1) as sp, \
                 tc.tile_pool(name="p7a", bufs=4) as ap, \
                 tc.tile_pool(name="p7p", bufs=2, space="PSUM") as psp, \
                 tc.tile_pool(name="p7f", bufs=1, space="PSUM") as psf:
                tr = sp.tile([TAGS, TAGS], f32, tag="tr")
                nc.sync.dma_start(tr[:], trans_d[:])
                ETp = sp.tile([TAGS, TAGS], bf16, tag="ETp")
                nln48 = sp.tile([TAGS, 1], f32, tag="nln48")
                nc.gpsimd.memset(nln48[:], -LN48)
                nc.scalar.activation(ETp[:], tr[:], ACT.Exp, bias=nln48[:, 0:1])
                stv = sp.tile([TAGS, 1], f32, tag="stv")
                nc.sync.dma_start(stv[:], start_d[:])
                env = sp.tile([TAGS, 1], f32, tag="env")
                nc.sync.dma_start(env[:], end_d[:])
                eend = sp.tile([TAGS, 1], bf16, tag="eend")
                nc.scalar.activation(eend[:], env[:], ACT.Exp)

                alpha = ap.tile([TAGS, BPC], bf16, tag="alpha")
                nc.scalar.activation(alpha[:], logitsT[:, 0:BPC], ACT.Exp,
                                     bias=stv[:, 0:1])
                for t in range(1, T if PH >= 7 else 1):
                    pm = psp.tile([TAGS, BPC], f32, space="PSUM", tag="pm")
                    nc.tensor.matmul(pm[:], lhsT=ETp[:], rhs=alpha[:],
                                     start=True, stop=True)
                    alpha = ap.tile([TAGS, BPC], bf16, tag="alpha")
                    nc.vector.tensor_tensor(out=alpha[:], in0=pm[:],
                                            in1=Esb[:, BPC * t:BPC * (t + 1)], op=MULT)
                pf = psf.tile([1, BPC], f32, space="PSUM", tag="pf")
                nc.tensor.matmul(pf[:], lhsT=eend[:], rhs=alpha[:],
                                 start=True, stop=True)
                logz = sp.tile([1, BPC], f32, tag="logz")
                nc.scalar.activation(logz[:], pf[:], ACT.Ln)
                nc.vector.tensor_scalar_add(logz[:], logz[:], float((T - 1) * LN48))
                nc.sync.dma_start(out_d[1:2, :], logz[:])

            # ================= P8: gold path score =========================
            with tc.tile_pool(name="p8", bufs=3) as sp, \
                 tc.tile_pool(name="p8a", bufs=1) as aw, \
                 tc.tile_pool(name="p8p", bufs=2, space="PSUM") as psp:
                accW = aw.tile([128, NM], f32, tag="accW")
                lfl = logits_t[:].rearrange("n k -> (n k)").unsqueeze(1)
                tfl = trans_d[:].rearrange("a b -> (a b)").unsqueeze(1)
                for m in range(NM if PH >= 8 else 0):
                    ie = sp.tile([128, 1], i32, tag="ie")
                    nc.sync.dma_start(ie[:], idx_emit[128 * m:128 * (m + 1), :])
                    it = sp.tile([128, 1], i32, tag="it")
                    nc.sync.dma_start(it[:], idx_trans[128 * m:128 * (m + 1), :])
                    ge = sp.tile([128, 1], f32, tag="ge")
                    nc.gpsimd.indirect_dma_start(
                        out=ge[:], out_offset=None, in_=lfl,
                        in_offset=bass.IndirectOffsetOnAxis(ap=ie[:, :1], axis=0))
                    gt = sp.tile([128, 1], f32, tag="gt")
                    nc.gpsimd.indirect_dma_start(
                        out=gt[:], out_offset=None, in_=tfl,
                        in_offset=bass.IndirectOffsetOnAxis(ap=it[:, :1], axis=0))
                    if m == NM - 1:
                        nc.vector.tensor_tensor(out=gt[:], in0=gt[:],
                                                in1=mask_sb[:], op=MULT)
                    nc.vector.tensor_tensor(out=accW[:, m:m + 1], in0=ge[:],
                                            in1=gt[:], op=ADD)
                se16 = sp.tile([16, 1], f32, tag="se16")
                ist = sp.tile([BPC, 1], i32, tag="ist")
                nc.sync.dma_start(ist[:], idx_start[:])
                nc.gpsimd.indirect_dma_start(
                    out=se16[0:8, :], out_offset=None,
                    in_=start_d[:].rearrange("a b -> (a b)").unsqueeze(1),
                    in_offset=bass.IndirectOffsetOnAxis(ap=ist[:, :1], axis=0))
                ien = sp.tile([BPC, 1], i32, tag="ien")
                nc.sync.dma_start(ien[:], idx_end[:])
                nc.gpsimd.indirect_dma_start(
                    out=se16[8:16, :], out_offset=None,
                    in_=end_d[:].rearrange("a b -> (a b)").unsqueeze(1),
                    in_offset=bass.IndirectOffsetOnAxis(ap=ien[:, :1], axis=0))
                s1 = psp.tile([NM, BPC], f32, space="PSUM", tag="s1")
                nc.tensor.matmul(s1[:], lhsT=accW[:], rhs=sel_sb[:],
                                 start=True, stop=True)
                s1s = sp.tile([NM, BPC], f32, tag="s1s")
                nc.scalar.copy(s1s[:], s1[:])
                s2 = psp.tile([1, BPC], f32, space="PSUM", tag="s2")
                nc.tensor.matmul(s2[:], lhsT=ones_f[0:NM, 0:1], rhs=s1s[:],
                                 start=True, stop=False)
                nc.tensor.matmul(s2[:], lhsT=se16[:], rhs=sel_sb[0:16, :],
                                 start=False, stop=True)
                joint = sp.tile([1, BPC], f32, tag="joint")
                nc.scalar.copy(joint[:], s2[:])
                nc.sync.dma_start(out_d[0:1, :], joint[:])

    _split_waits(nc, maxw=int(__import__("os").environ.get("KMAXW", "1")))
    return nc


def _split_waits(nc, maxw=2):
    """This container's walrus rejects instructions carrying more than a
    couple of semaphore waits. Hoist extras onto preceding same-engine
    NoOps (engines execute their stream in order, so this preserves the
    happens-before)."""
    import concourse.mybir as mybir
    import bass_rust
    compute_ops = {"Matmult", "Activation", "TensorTensor", "TensorScalar",
                   "TensorCopy", "TensorReduce", "Memset", "Iota",
                   "AffineSelect", "TensorTensorScan", "Select"}
    n_added = 0
    for fn in nc.m.functions:
        for blk in fn.blocks:
            insts = list(blk.instructions)
            out = []
            dirty = False
            for inst in insts:
                mw = 2 if (maxw == 0 and str(inst.opcode) in compute_ops) else max(1, maxw)
                si = inst.sync_info
                if si is not None and len(si.on_wait) > mw:
                    waits = list(si.on_wait)
                    extra, keep = waits[:-mw], waits[-mw:]
                    for i in range(0, len(extra), mw):
                        nop = mybir.InstNoOp(
                            name=f"{inst.name}_hw{i}", ins=[], outs=[])
                        nop.engine = inst.engine
                        nop.sync_info = bass_rust.SyncInfo(
                            on_wait=extra[i:i + mw], on_update=[])
                        out.append(nop)
                        n_added += 1
                    inst.sync_info = bass_rust.SyncInfo(
                        on_wait=keep, on_update=list(si.on_update))
                    dirty = True
                out.append(inst)
            if dirty:
                blk.instructions = out
    return n_added


def _prep_inputs(inputs):
    import ml_dtypes
    bf = ml_dtypes.bfloat16

    g = {k: np.asarray(v) for k, v in inputs.items()}
    seq = g["sequences"].astype(np.int64)
    tags = g["tags"].astype(np.int64)

    p1 = _gate_perm(H1)
    p2 = _gate_perm(H2)

    shared = {}
    shared["embed_bf"] = np.ascontiguousarray(g["embed_table"].astype(bf))
    for d, sfx in enumerate(["1f", "1b"]):
        wih = g["w_ih" + sfx][p1]  # [2048, 512] permuted rows
        whh = g["w_hh" + sfx][p1]
        b = (g["b_ih" + sfx] + g["b_hh" + sfx])[p1]
        shared.setdefault("wihT1", np.zeros((2, EMBED, 4 * H1), bf))[d] = wih.T.astype(bf)
        shared.setdefault("whhT1", np.zeros((2, H1, 4 * H1), bf))[d] = whh.T.astype(bf)
        shared.setdefault("bias1", np.zeros((2, 1, 4 * H1), bf))[d] = b.astype(bf)[None]
    for d, sfx in enumerate(["2f", "2b"]):
        wih = g["w_ih" + sfx][p2]
        whh = g["w_hh" + sfx][p2]
        b = (g["b_ih" + sfx] + g["b_hh" + sfx])[p2]
        shared.setdefault("wihT2", np.zeros((2, HID, 4 * H2), bf))[d] = wih.T.astype(bf)
        shared.setdefault("whhT2", np.zeros((2, H2, 4 * H2), bf))[d] = whh.T.astype(bf)
        shared.setdefault("bias2", np.zeros((2, 1, 4 * H2), bf))[d] = b.astype(bf)[None]
    shared["linWT"] = np.ascontiguousarray(g["lin_w"].T.astype(bf))
    shared["lin_b"] = g["lin_b"].astype(bf)[None, :]
    shared["transitions"] = g["transitions"].astype(np.float32)
    shared["start_trans"] = g["start_trans"].astype(np.float32)[:, None]
    shared["end_trans"] = g["end_trans"].astype(np.float32)[:, None]
    sel = (np.arange(128)[:, None] % 8 == np.arange(8)[None, :]).astype(np.float32)
    shared["sel"] = sel
    mask = np.ones((128, 1), np.float32)
    mask[120:128] = 0.0
    shared["mask_last"] = mask

    in_maps = []
    for c in range(8):
        b0 = c * BPC
        m = dict(shared)
        sl = slice(b0, b0 + BPC)
        seq_c = seq[sl]  # [8, T]
        tags_c = tags[sl]
        tok_seq = seq_c.T.reshape(NTOK).astype(np.int32)  # token-major (t, b)
        tok_tags = tags_c.T.reshape(NTOK).astype(np.int64)
        m["seq_tok"] = tok_seq[:, None]
        m["idx_emit"] = (np.arange(NTOK, dtype=np.int64) * TAGS + tok_tags).astype(np.int32)[:, None]
        nxt = np.concatenate([tok_tags[BPC:], np.zeros(BPC, np.int64)])
        it = tok_tags * TAGS + nxt
        it[-BPC:] = 0
        m["idx_trans"] = it.astype(np.int32)[:, None]
        m["idx_start"] = tags_c[:, 0].astype(np.int32)[:, None]
        m["idx_end"] = tags_c[:, T - 1].astype(np.int32)[:, None]

        c0l1 = np.zeros((2, 128, 128), np.float32)
        h0l1 = np.zeros((2, H1, BPC), np.float32)
        for d in range(2):
            cc = g["c0"][d, sl]  # [8, 512]
            hh = g["h0"][d, sl]
            for j in range(4):
                c0l1[d, 32 * j:32 * j + BPC, :] = cc[:, 128 * j:128 * (j + 1)]
            h0l1[d] = hh.T
        m["c0_l1"] = c0l1
        m["h0T_l1"] = h0l1.astype(bf)
        c0l2 = np.zeros((128, 128), np.float32)
        h0l2 = np.zeros((2, H2, BPC), np.float32)
        for d in range(2):
            cc = g["c1"][d, sl]  # [8, 256]
            hh = g["h1"][d, sl]
            for j in range(2):
                q = 2 * d + j
                c0l2[32 * q:32 * q + BPC, :] = cc[:, 128 * j:128 * (j + 1)]
            h0l2[d] = hh.T
        m["c0_l2"] = c0l2
        m["h0T_l2"] = h0l2.astype(bf)
        in_maps.append(m)
    return in_maps


def kernel(**inputs) -> np.ndarray:
    import time
    from concourse.bass_utils import run_bass_kernel_spmd

    if "nc" not in _CACHE:
        _CACHE["nc"] = _build_program()
    nc = _CACHE["nc"]

    in_maps = _prep_inputs(inputs)
    res = None
    for attempt in range(3):
        try:
            res = run_bass_kernel_spmd(nc, in_maps, core_ids=list(range(8)))
            break
        except Exception:
            # transient NRT_EXEC_UNIT_UNRECOVERABLE after wedged runs —
            # observed to recover after ~60s
            if attempt == 2:
                raise
            time.sleep(60)
    loss = np.float64(0.0)
    for r in res.results:
        o = r["out"].astype(np.float64)  # [2, 8]: joint, logz
        loss += np.sum(o[0] - o[1])
    return np.float32(-loss)



# revision 12
# speedup vs baseline: 2.7163x; 2.7163x over previous
"""BiLSTM-CRF forward loss on 8 Trainium2 cores (batch-parallel SPMD).

Layout/sharding summary (per core, b=8 examples of B=64):
- embedding gather -> x^T (PE transposes) kept in SBUF
- LSTM scans run W-stationary: gates live TRANSPOSED in PSUM as
  [128p = gate-unit-in-chunk, cols = (chunk, step-in-group, batch)].
  Per 4-step group: one bias selector matmul + input-projection matmuls
  (4 steps wide); per step: tiny [128,8]-output recurrent matmuls.
  Elementwise cell update on full-128-partition tiles; h written
  straight into a transposed SBUF stash (no per-step transposes).
- L2 scan reads the L1 stash directly (no DRAM round trip).
- linear -> logitsT [48, T*8] and logits [T*8, 48]
- CRF forward pass in exp-domain: alpha_t = (expT^T @ alpha) * exp(emit_t)
- gold path score via indirect-DMA gathers + selector matmuls
Outputs per core: [2, 8] fp32 (row0 joint, row1 logZ). Host sums
-(joint - logZ) over all 64 examples.
"""

import numpy as np

B, T, VOCAB, EMBED, HID, TAGS = 64, 512, 30000, 512, 1024, 48
H1, H2 = HID // 2, HID // 4  # 512, 256
BPC = B // 8  # batch per core = 8
NTOK = T * BPC  # 4096 tokens per core
LN48 = float(np.log(48.0))
NC1, KB1 = 16, 4  # L1 gate chunks (2048/128), hidden blocks (512/128)
NC2, KB2 = 8, 2   # L2 gate chunks (1024/128), hidden blocks (256/128)
KI2 = 8           # L2 input blocks (1024/128)

_CACHE = {}


def _build_program():
    import concourse.bass as bass
    import concourse.tile as tile
    import concourse.mybir as mybir
    from concourse.vector_clock import ScopedClock, VectorClock
    from concourse.masks import make_identity

    def _patched_drain_and_barrier(self, tick_clock, wait_clock):
        # This container's walrus rejects >2 sem waits on one CTRL
        # instruction; split the kernel-tail drain waits into per-proc
        # NOP waits on the same (in-order) SP queue.
        vc = tick_clock.global_clock
        n = len(vc)
        for p in range(n):
            t = vc[p]
            if t > 0:
                vec = [0] * n
                vec[p] = t
                nop = self.nc.sync.nop()
                wait_clock.add_sem_waits(nop.ins, ScopedClock({None: VectorClock(vec)}))
        self.nc.sync.drain()
        self.nc.all_engine_barrier()
        popped = self.nc._tile_sem_poison_stack.pop()
        assert popped is self._sem_poison
        self.nc.clear_and_free_semaphores(list(self.sems.allocated().values()))
        self.nc.all_engine_barrier()

    tile.TileContext._drain_and_barrier = _patched_drain_and_barrier

    f32 = mybir.dt.float32
    bf16 = mybir.dt.bfloat16
    i32 = mybir.dt.int32
    ACT = mybir.ActivationFunctionType
    ADD = mybir.AluOpType.add
    MULT = mybir.AluOpType.mult

    nc = bass.Bass()
    PH = int(__import__("os").environ.get("KPHASES", "99"))

    # ---------------- inputs ----------------
    def din(name, shape, dt=f32):
        return nc.dram_tensor(name, shape, dt, kind="ExternalInput")

    embed_bf = din("embed_bf", [VOCAB, EMBED], bf16)
    seq_tok = din("seq_tok", [NTOK, 1], i32)
    idx_emit = din("idx_emit", [NTOK, 1], i32)
    idx_trans = din("idx_trans", [NTOK, 1], i32)
    idx_start = din("idx_start", [BPC, 1], i32)
    idx_end = din("idx_end", [BPC, 1], i32)
    w1x_d = din("w1x", [2, EMBED, 4 * H1], bf16)   # W_ih1^T, perm'd gate cols
    w1h_d = din("w1h", [2, H1, 4 * H1], bf16)      # W_hh1^T
    b1_d = din("b1", [2, NC1, 128], bf16)          # bias rows per chunk
    w2x_d = din("w2x", [2, 2 * H1, 4 * H2], bf16)
    w2h_d = din("w2h", [2, H2, 4 * H2], bf16)
    b2_d = din("b2", [2, NC2, 128], bf16)
    sel16_d = din("sel16", [NC1, 512], bf16)       # chunk-selector, 32-wide
    sel8_d = din("sel8", [NC2, 256], bf16)
    linWT = din("linWT", [H2 * 2, TAGS], bf16)
    lin_b = din("lin_b", [1, TAGS], bf16)
    h0l1_d = din("h0l1", [2, 128, 8 * KB1], bf16)  # (p, b, j)
    c0l1_d = din("c0l1", [2, 128, 8 * KB1])
    h0l2_d = din("h0l2", [2, 128, 8 * KB2], bf16)
    c0l2_d = din("c0l2", [2, 128, 8 * KB2])
    trans_d = din("transitions", [TAGS, TAGS])
    start_d = din("start_trans", [TAGS, 1])
    end_d = din("end_trans", [TAGS, 1])
    sel_d = din("sel", [128, BPC])  # sel[p, b] = (p % 8 == b), fp32
    mask_d = din("mask_last", [128, 1])  # 1.0 except rows 120..127 -> 0.0

    out_d = nc.dram_tensor("out", [2, BPC], f32, kind="ExternalOutput")

    NM = NTOK // 128  # 32 token chunks

    with tile.TileContext(nc) as tc:
        with tc.tile_pool(name="dram", bufs=1, space="DRAM") as dpool, \
             tc.tile_pool(name="const", bufs=1) as cpool, \
             tc.tile_pool(name="persist", bufs=1) as ppool:

            logits_t = dpool.tile([NTOK, TAGS], f32)

            ones_bf = cpool.tile([1, 512], bf16)
            nc.gpsimd.memset(ones_bf[:], 1.0)
            ones_f = cpool.tile([128, 1], f32)
            nc.gpsimd.memset(ones_f[:], 1.0)
            id128 = cpool.tile([128, 128], bf16)
            make_identity(nc, id128[:])
            sel_sb = cpool.tile([128, BPC], f32)
            nc.sync.dma_start(sel_sb[:], sel_d[:])
            mask_sb = cpool.tile([128, 1], f32)
            nc.sync.dma_start(mask_sb[:], mask_d[:])

            # persistent state across phases
            stash1 = [ppool.tile([128, KB1 * NTOK], bf16, tag=f"st1_{d}",
                                 name=f"st1_{d}") for d in range(2)]
            stash2 = [ppool.tile([128, KB2 * NTOK], bf16, tag=f"st2_{d}",
                                 name=f"st2_{d}") for d in range(2)]


            # ============ P1 (embed gather -> xT sbuf) + P3 (L1 scan) ======
            with tc.tile_pool(name="w13", bufs=1) as wp, \
                 tc.tile_pool(name="p1", bufs=2) as sp1, \
                 tc.tile_pool(name="p1p", bufs=4, space="PSUM") as ps1, \
                 tc.tile_pool(name="st3", bufs=1) as stp, \
                 tc.tile_pool(name="p3", bufs=4) as sp3, \
                 tc.tile_pool(name="p3g", bufs=2, space="PSUM") as psg:

                xT = [wp.tile([128, NTOK], bf16, tag=f"xT{k}", name=f"xT{k}")
                      for k in range(KB1)]
                # interleave low/high chunks so both scan directions start early
                ms = []
                for a in range(NM // 2):
                    ms += [a, NM - 1 - a]
                for m in (ms if PH >= 1 else []):
                    idx = sp1.tile([128, 1], i32, tag="idx")
                    nc.sync.dma_start(idx[:], seq_tok[128 * m:128 * (m + 1), :])
                    xg = sp1.tile([128, EMBED], bf16, tag="xg")
                    nc.gpsimd.indirect_dma_start(
                        out=xg[:], out_offset=None, in_=embed_bf[:],
                        in_offset=bass.IndirectOffsetOnAxis(ap=idx[:, :1], axis=0))
                    for e in range(EMBED // 128):
                        pt = ps1.tile([128, 128], bf16, space="PSUM", tag="pt")
                        nc.tensor.transpose(out=pt[:], in_=xg[:, 128 * e:128 * (e + 1)],
                                            identity=id128[:])
                        if e % 2 == 0:
                            nc.vector.tensor_copy(xT[e][:, 128 * m:128 * (m + 1)], pt[:])
                        else:
                            nc.scalar.copy(xT[e][:, 128 * m:128 * (m + 1)], pt[:])

                # ---- L1 weights ----
                w1x = {}
                w1h = {}
                for d in range(2):
                    for k in range(KB1):
                        w = wp.tile([128, 4 * H1], bf16, tag=f"w1x_{d}_{k}")
                        nc.sync.dma_start(w[:], w1x_d[d, 128 * k:128 * (k + 1), :])
                        w1x[(d, k)] = w
                        w = wp.tile([128, 4 * H1], bf16, tag=f"w1h_{d}_{k}")
                        nc.sync.dma_start(w[:], w1h_d[d, 128 * k:128 * (k + 1), :])
                        w1h[(d, k)] = w
                b1t = []
                for d in range(2):
                    b = wp.tile([NC1, 128], bf16, tag=f"b1_{d}")
                    nc.sync.dma_start(b[:], b1_d[d])
                    b1t.append(b)
                sel16 = wp.tile([NC1, 512], bf16, tag="sel16")
                nc.sync.dma_start(sel16[:], sel16_d[:])

                cS = {}
                h0t1 = []
                for d in range(2):
                    for par in range(2):
                        c = stp.tile([128, 8 * KB1], f32, tag=f"c1_{d}_{par}")
                        cS[(d, par)] = c
                    nc.sync.dma_start(cS[(d, 0)][:], c0l1_d[d])
                    h0 = stp.tile([128, 8 * KB1], bf16, tag=f"h01_{d}")
                    nc.sync.dma_start(h0[:], h0l1_d[d])
                    h0t1.append(h0)

                # ---- L1 scan: 4-step groups ----
                for g in range(T // 4 if PH >= 3 else 0):
                    s0 = 4 * g
                    pgs = []
                    for d in range(2):
                        pg = psg.tile([128, 512], f32, space="PSUM", tag=f"pg{d}")
                        pgs.append(pg)
                        tb = s0 if d == 0 else T - 4 - s0  # lowest t in group
                        nc.tensor.matmul(pg[:], lhsT=b1t[d][:], rhs=sel16[:],
                                         start=True, stop=False,
                                         skip_group_check=True)
                        for c in range(NC1):
                            for k in range(KB1):
                                nc.tensor.matmul(
                                    pg[:, 32 * c:32 * c + 32],
                                    lhsT=w1x[(d, k)][:, 128 * c:128 * (c + 1)],
                                    rhs=xT[k][:, 8 * tb:8 * tb + 32],
                                    start=False, stop=False,
                                    skip_group_check=True)
                    for i in range(4):
                        s = s0 + i
                        for d in range(2):
                            t = s if d == 0 else T - 1 - s
                            tp = i if d == 0 else 3 - i
                            pg = pgs[d]
                            for c in range(NC1):
                                for k in range(KB1):
                                    if s == 0:
                                        rhs = h0t1[d][:, 8 * k:8 * k + 8]
                                    else:
                                        tprev = (s - 1) if d == 0 else (T - s)
                                        o = k * NTOK + 8 * tprev
                                        rhs = stash1[d][:, o:o + 8]
                                    nc.tensor.matmul(
                                        pg[:, 32 * c + 8 * tp:32 * c + 8 * tp + 8],
                                        lhsT=w1h[(d, k)][:, 128 * c:128 * (c + 1)],
                                        rhs=rhs, start=False, stop=(k == KB1 - 1),
                                        skip_group_check=True)
                            pgv = pg[:].rearrange("p (c q) -> p c q", q=32)
                            sfo = sp3.tile([128, 96], bf16, tag=f"sfo{d}")
                            nc.scalar.activation(
                                sfo[:].rearrange("p (c q) -> p c q", q=8),
                                pgv[:, 0:12, 8 * tp:8 * tp + 8], ACT.Sigmoid)
                            tg = sp3.tile([128, 32], bf16, tag=f"tg{d}")
                            nc.scalar.activation(
                                tg[:].rearrange("p (c q) -> p c q", q=8),
                                pgv[:, 12:16, 8 * tp:8 * tp + 8], ACT.Tanh)
                            c_old = cS[(d, s % 2)]
                            c_new = cS[(d, (s + 1) % 2)]
                            t1 = sp3.tile([128, 32], f32, tag=f"t1{d}")
                            nc.vector.tensor_tensor(out=t1[:], in0=sfo[:, 32:64],
                                                    in1=c_old[:], op=MULT)
                            t2 = sp3.tile([128, 32], f32, tag=f"t2{d}")
                            nc.vector.tensor_tensor(out=t2[:], in0=sfo[:, 0:32],
                                                    in1=tg[:], op=MULT)
                            nc.vector.tensor_tensor(out=c_new[:], in0=t1[:],
                                                    in1=t2[:], op=ADD)
                            thc = sp3.tile([128, 32], bf16, tag=f"thc{d}")
                            nc.scalar.activation(thc[:], c_new[:], ACT.Tanh)
                            hout = stash1[d][:].rearrange(
                                "p (b n) -> p b n", b=KB1)[:, :, 8 * t:8 * t + 8]
                            nc.vector.tensor_tensor(
                                out=hout,
                                in0=sfo[:, 64:96].rearrange("p (b q) -> p b q", q=8),
                                in1=thc[:].rearrange("p (b q) -> p b q", q=8),
                                op=MULT)

            # ================= P5: L2 scan ================================
            with tc.tile_pool(name="w5", bufs=1) as wp, \
                 tc.tile_pool(name="st5", bufs=1) as stp, \
                 tc.tile_pool(name="p5", bufs=4) as sp5, \
                 tc.tile_pool(name="p5g", bufs=2, space="PSUM") as psg:
                w2x = {}
                w2h = {}
                for d in range(2):
                    for k in range(KI2):
                        w = wp.tile([128, 4 * H2], bf16, tag=f"w2x_{d}_{k}")
                        nc.sync.dma_start(w[:], w2x_d[d, 128 * k:128 * (k + 1), :])
                        w2x[(d, k)] = w
                    for k in range(KB2):
                        w = wp.tile([128, 4 * H2], bf16, tag=f"w2h_{d}_{k}")
                        nc.sync.dma_start(w[:], w2h_d[d, 128 * k:128 * (k + 1), :])
                        w2h[(d, k)] = w
                b2t = []
                for d in range(2):
                    b = wp.tile([NC2, 128], bf16, tag=f"b2_{d}")
                    nc.sync.dma_start(b[:], b2_d[d])
                    b2t.append(b)
                sel8 = wp.tile([NC2, 256], bf16, tag="sel8")
                nc.sync.dma_start(sel8[:], sel8_d[:])

                c2 = {}
                h0t2 = []
                for d in range(2):
                    for par in range(2):
                        c = stp.tile([128, 8 * KB2], f32, tag=f"c2_{d}_{par}")
                        c2[(d, par)] = c
                    nc.sync.dma_start(c2[(d, 0)][:], c0l2_d[d])
                    h0 = stp.tile([128, 8 * KB2], bf16, tag=f"h02_{d}")
                    nc.sync.dma_start(h0[:], h0l2_d[d])
                    h0t2.append(h0)

                for g in range(T // 4 if PH >= 5 else 0):
                    s0 = 4 * g
                    pg = psg.tile([128, 512], f32, space="PSUM", tag="pg5")
                    for d in range(2):
                        base = 256 * d
                        tb = s0 if d == 0 else T - 4 - s0
                        nc.tensor.matmul(pg[:, base:base + 256], lhsT=b2t[d][:],
                                         rhs=sel8[:], start=True, stop=False,
                                         skip_group_check=True)
                        for c in range(NC2):
                            for k in range(KI2):
                                src = stash1[k // KB1][
                                    :, (k % KB1) * NTOK + 8 * tb:
                                    (k % KB1) * NTOK + 8 * tb + 32]
                                nc.tensor.matmul(
                                    pg[:, base + 32 * c:base + 32 * c + 32],
                                    lhsT=w2x[(d, k)][:, 128 * c:128 * (c + 1)],
                                    rhs=src, start=False, stop=False,
                                    skip_group_check=True)
                    for i in range(4):
                        s = s0 + i
                        for d in range(2):
                            base = 256 * d
                            t = s if d == 0 else T - 1 - s
                            tp = i if d == 0 else 3 - i
                            for c in range(NC2):
                                for k in range(KB2):
                                    if s == 0:
                                        rhs = h0t2[d][:, 8 * k:8 * k + 8]
                                    else:
                                        tprev = (s - 1) if d == 0 else (T - s)
                                        o = k * NTOK + 8 * tprev
                                        rhs = stash2[d][:, o:o + 8]
                                    nc.tensor.matmul(
                                        pg[:, base + 32 * c + 8 * tp:
                                           base + 32 * c + 8 * tp + 8],
                                        lhsT=w2h[(d, k)][:, 128 * c:128 * (c + 1)],
                                        rhs=rhs, start=False, stop=(k == KB2 - 1),
                                        skip_group_check=True)
                            pgv = pg[:].rearrange("p (h c q) -> p h c q", h=2, q=32)
                            sfo = sp5.tile([128, 48], bf16, tag=f"sfo5{d}")
                            nc.scalar.activation(
                                sfo[:].rearrange("p (c q) -> p c q", q=8),
                                pgv[:, d, 0:6, 8 * tp:8 * tp + 8], ACT.Sigmoid)
                            tg = sp5.tile([128, 16], bf16, tag=f"tg5{d}")
                            nc.scalar.activation(
                                tg[:].rearrange("p (c q) -> p c q", q=8),
                                pgv[:, d, 6:8, 8 * tp:8 * tp + 8], ACT.Tanh)
                            c_old = c2[(d, s % 2)]
                            c_new = c2[(d, (s + 1) % 2)]
                            t1 = sp5.tile([128, 16], f32, tag=f"t15{d}")
                            nc.vector.tensor_tensor(out=t1[:], in0=sfo[:, 16:32],
                                                    in1=c_old[:], op=MULT)
                            t2 = sp5.tile([128, 16], f32, tag=f"t25{d}")
                            nc.vector.tensor_tensor(out=t2[:], in0=sfo[:, 0:16],
                                                    in1=tg[:], op=MULT)
                            nc.vector.tensor_tensor(out=c_new[:], in0=t1[:],
                                                    in1=t2[:], op=ADD)
                            thc = sp5.tile([128, 16], bf16, tag=f"thc5{d}")
                            nc.scalar.activation(thc[:], c_new[:], ACT.Tanh)
                            hout = stash2[d][:].rearrange(
                                "p (b n) -> p b n", b=KB2)[:, :, 8 * t:8 * t + 8]
                            nc.vector.tensor_tensor(
                                out=hout,
                                in0=sfo[:, 32:48].rearrange("p (b q) -> p b q", q=8),
                                in1=thc[:].rearrange("p (b q) -> p b q", q=8),
                                op=MULT)

            # ================= P6: linear -> Esb + logits ==================
            epool_cm = tc.tile_pool(name="e67", bufs=1)
            epool = epool_cm.__enter__()
            Esb = epool.tile([TAGS, NTOK], bf16, tag="Esb")
            with tc.tile_pool(name="w6", bufs=1) as wp, \
                 tc.tile_pool(name="p6", bufs=3) as sp, \
                 tc.tile_pool(name="p6p", bufs=4, space="PSUM") as psp:
                lw = [wp.tile([128, TAGS], bf16, tag=f"lw{q}", name=f"lw{q}")
                      for q in range(4)]
                for q in range(4):
                    nc.sync.dma_start(lw[q][:], linWT[128 * q:128 * (q + 1), :])
                lb = wp.tile([1, TAGS], bf16, tag="lb")
                nc.sync.dma_start(lb[:], lin_b[:])
                # logitsT [48, NTOK] (+ exp) straight from the h2 stash
                for n in range(NTOK // 512 if PH >= 6 else 0):
                    pg = psp.tile([TAGS, 512], f32, space="PSUM", tag="pl")
                    for q in range(4):
                        d, bq = q // 2, q % 2
                        nc.tensor.matmul(
                            pg[:], lhsT=lw[q][:],
                            rhs=stash2[d][:, bq * NTOK + 512 * n:
                                          bq * NTOK + 512 * (n + 1)],
                            start=(q == 0), stop=False)
                    nc.tensor.matmul(pg[:], lhsT=lb[0:1, :], rhs=ones_bf[0:1, :],
                                     start=False, stop=True)
                    nc.scalar.activation(Esb[:, 512 * n:512 * (n + 1)],
                                         pg[:], ACT.Exp)
                # logits [NTOK, 48] to DRAM for gathers
                for m in range(NM if PH >= 6 else 0):
                    pg = psp.tile([128, TAGS], f32, space="PSUM", tag="pl2")
                    for q in range(4):
                        d, bq = q // 2, q % 2
                        nc.tensor.matmul(
                            pg[:], lhsT=stash2[d][:, bq * NTOK + 128 * m:
                                                  bq * NTOK + 128 * (m + 1)],
                            rhs=lw[q][:], start=(q == 0), stop=False)
                    nc.tensor.matmul(pg[:], lhsT=ones_bf[0:1, 0:128], rhs=lb[0:1, :],
                                     start=False, stop=True)
                    ls = sp.tile([128, TAGS], f32, tag="ls")
                    nc.scalar.copy(ls[:], pg[:])
                    nc.sync.dma_start(logits_t[128 * m:128 * (m + 1), :], ls[:])

            # ================= P7: CRF partition (exp domain) ==============
            with tc.tile_pool(name="p7", bufs=1) as sp, \
                 tc.tile_pool(name="p7a", bufs=4) as ap, \
                 tc.tile_pool(name="p7p", bufs=2, space="PSUM") as psp, \
                 tc.tile_pool(name="p7f", bufs=1, space="PSUM") as psf:
                tr = sp.tile([TAGS, TAGS], f32, tag="tr")
                nc.sync.dma_start(tr[:], trans_d[:])
                ETp = sp.tile([TAGS, TAGS], bf16, tag="ETp")
                nln48 = sp.tile([TAGS, 1], f32, tag="nln48")
                nc.gpsimd.memset(nln48[:], -LN48)
                nc.scalar.activation(ETp[:], tr[:], ACT.Exp, bias=nln48[:, 0:1])
                stv = sp.tile([TAGS, 1], f32, tag="stv")
                nc.sync.dma_start(stv[:], start_d[:])
                estart = sp.tile([TAGS, 1], f32, tag="estart")
                nc.scalar.activation(estart[:], stv[:], ACT.Exp)
                env = sp.tile([TAGS, 1], f32, tag="env")
                nc.sync.dma_start(env[:], end_d[:])
                eend = sp.tile([TAGS, 1], bf16, tag="eend")
                nc.scalar.activation(eend[:], env[:], ACT.Exp)

                alpha = ap.tile([TAGS, BPC], bf16, tag="alpha")
                nc.vector.tensor_scalar_mul(alpha[:], Esb[:, 0:BPC],
                                            estart[:, 0:1])
                for t in range(1, T if PH >= 7 else 1):
                    pm = psp.tile([TAGS, BPC], f32, space="PSUM", tag="pm")
                    nc.tensor.matmul(pm[:], lhsT=ETp[:], rhs=alpha[:],
                                     start=True, stop=True)
                    alpha = ap.tile([TAGS, BPC], bf16, tag="alpha")
                    nc.vector.tensor_tensor(out=alpha[:], in0=pm[:],
                                            in1=Esb[:, BPC * t:BPC * (t + 1)], op=MULT)
                pf = psf.tile([1, BPC], f32, space="PSUM", tag="pf")
                nc.tensor.matmul(pf[:], lhsT=eend[:], rhs=alpha[:],
                                 start=True, stop=True)
                logz = sp.tile([1, BPC], f32, tag="logz")
                nc.scalar.activation(logz[:], pf[:], ACT.Ln)
                nc.vector.tensor_scalar_add(logz[:], logz[:], float((T - 1) * LN48))
                nc.sync.dma_start(out_d[1:2, :], logz[:])
            epool_cm.__exit__(None, None, None)

            # ================= P8: gold path score =========================
            with tc.tile_pool(name="p8", bufs=3) as sp, \
                 tc.tile_pool(name="p8a", bufs=1) as aw, \
                 tc.tile_pool(name="p8p", bufs=2, space="PSUM") as psp:
                accW = aw.tile([128, NM], f32, tag="accW")
                lfl = logits_t[:].rearrange("n k -> (n k)").unsqueeze(1)
                tfl = trans_d[:].rearrange("a b -> (a b)").unsqueeze(1)
                for m in range(NM if PH >= 8 else 0):
                    ie = sp.tile([128, 1], i32, tag="ie")
                    nc.sync.dma_start(ie[:], idx_emit[128 * m:128 * (m + 1), :])
                    it = sp.tile([128, 1], i32, tag="it")
                    nc.sync.dma_start(it[:], idx_trans[128 * m:128 * (m + 1), :])
                    ge = sp.tile([128, 1], f32, tag="ge")
                    nc.gpsimd.indirect_dma_start(
                        out=ge[:], out_offset=None, in_=lfl,
                        in_offset=bass.IndirectOffsetOnAxis(ap=ie[:, :1], axis=0))
                    gt = sp.tile([128, 1], f32, tag="gt")
                    nc.gpsimd.indirect_dma_start(
                        out=gt[:], out_offset=None, in_=tfl,
                        in_offset=bass.IndirectOffsetOnAxis(ap=it[:, :1], axis=0))
                    if m == NM - 1:
                        nc.vector.tensor_tensor(out=gt[:], in0=gt[:],
                                                in1=mask_sb[:], op=MULT)
                    nc.vector.tensor_tensor(out=accW[:, m:m + 1], in0=ge[:],
                                            in1=gt[:], op=ADD)
                se16 = sp.tile([16, 1], f32, tag="se16")
                ist = sp.tile([BPC, 1], i32, tag="ist")
                nc.sync.dma_start(ist[:], idx_start[:])
                nc.gpsimd.indirect_dma_start(
                    out=se16[0:8, :], out_offset=None,
                    in_=start_d[:].rearrange("a b -> (a b)").unsqueeze(1),
                    in_offset=bass.IndirectOffsetOnAxis(ap=ist[:, :1], axis=0))
                ien = sp.tile([BPC, 1], i32, tag="ien")
                nc.sync.dma_start(ien[:], idx_end[:])
                nc.gpsimd.indirect_dma_start(
                    out=se16[8:16, :], out_offset=None,
                    in_=end_d[:].rearrange("a b -> (a b)").unsqueeze(1),
                    in_offset=bass.IndirectOffsetOnAxis(ap=ien[:, :1], axis=0))
                s1 = psp.tile([NM, BPC], f32, space="PSUM", tag="s1")
                nc.tensor.matmul(s1[:], lhsT=accW[:], rhs=sel_sb[:],
                                 start=True, stop=True)
                s1s = sp.tile([NM, BPC], f32, tag="s1s")
                nc.scalar.copy(s1s[:], s1[:])
                s2 = psp.tile([1, BPC], f32, space="PSUM", tag="s2")
                nc.tensor.matmul(s2[:], lhsT=ones_f[0:NM, 0:1], rhs=s1s[:],
                                 start=True, stop=False)
                nc.tensor.matmul(s2[:], lhsT=se16[:], rhs=sel_sb[0:16, :],
                                 start=False, stop=True)
                joint = sp.tile([1, BPC], f32, tag="joint")
                nc.scalar.copy(joint[:], s2[:])
                nc.sync.dma_start(out_d[0:1, :], joint[:])

    _split_waits(nc, maxw=int(__import__("os").environ.get("KMAXW", "1")))
    return nc


def _split_waits(nc, maxw=2):
    """This container's walrus rejects instructions carrying more than a
    couple of semaphore waits. Hoist extras onto preceding same-engine
    NoOps (engines execute their stream in order, so this preserves the
    happens-before)."""
    import concourse.mybir as mybir
    import bass_rust
    compute_ops = {"Matmult", "Activation", "TensorTensor", "TensorScalar",
                   "TensorCopy", "TensorReduce", "Memset", "Iota",
                   "AffineSelect", "TensorTensorScan", "Select"}
    n_added = 0
    for fn in nc.m.functions:
        for blk in fn.blocks:
            insts = list(blk.instructions)
            out = []
            dirty = False
            for inst in insts:
                mw = 2 if (maxw == 0 and str(inst.opcode) in compute_ops) else max(1, maxw)
                si = inst.sync_info
                if si is not None and len(si.on_wait) > mw:
                    waits = list(si.on_wait)
                    extra, keep = waits[:-mw], waits[-mw:]
                    for i in range(0, len(extra), mw):
                        nop = mybir.InstNoOp(
                            name=f"{inst.name}_hw{i}", ins=[], outs=[])
                        nop.engine = inst.engine
                        nop.sync_info = bass_rust.SyncInfo(
                            on_wait=extra[i:i + mw], on_update=[])
                        out.append(nop)
                        n_added += 1
                    inst.sync_info = bass_rust.SyncInfo(
                        on_wait=keep, on_update=list(si.on_update))
                    dirty = True
                out.append(inst)
            if dirty:
                blk.instructions = out
    return n_added


def _prep_inputs(inputs):
    import ml_dtypes
    bf = ml_dtypes.bfloat16

    g = {k: np.asarray(v) for k, v in inputs.items()}
    seq = g["sequences"].astype(np.int64)
    tags = g["tags"].astype(np.int64)

    # gate-chunk permutation: chunk c = 4*t_idx + b (L1) / 2*t_idx + b (L2),
    # t_idx order (i, f, o, g) from pytorch row order (i, f, g, o)
    def gate_perm(h, nb):
        return np.concatenate([
            np.arange(gs * h + 128 * b, gs * h + 128 * b + 128)
            for gs in [0, 1, 3, 2] for b in range(nb)])

    p1 = gate_perm(H1, KB1)
    p2 = gate_perm(H2, KB2)

    shared = {}
    shared["embed_bf"] = np.ascontiguousarray(g["embed_table"].astype(bf))
    for d, sfx in enumerate(["1f", "1b"]):
        wih = g["w_ih" + sfx][p1]  # [2048, 512] permuted rows
        whh = g["w_hh" + sfx][p1]
        b = (g["b_ih" + sfx] + g["b_hh" + sfx])[p1]
        shared.setdefault("w1x", np.zeros((2, EMBED, 4 * H1), bf))[d] = wih.T.astype(bf)
        shared.setdefault("w1h", np.zeros((2, H1, 4 * H1), bf))[d] = whh.T.astype(bf)
        shared.setdefault("b1", np.zeros((2, NC1, 128), bf))[d] = \
            b.astype(bf).reshape(NC1, 128)
    for d, sfx in enumerate(["2f", "2b"]):
        wih = g["w_ih" + sfx][p2]
        whh = g["w_hh" + sfx][p2]
        b = (g["b_ih" + sfx] + g["b_hh" + sfx])[p2]
        shared.setdefault("w2x", np.zeros((2, HID, 4 * H2), bf))[d] = wih.T.astype(bf)
        shared.setdefault("w2h", np.zeros((2, H2, 4 * H2), bf))[d] = whh.T.astype(bf)
        shared.setdefault("b2", np.zeros((2, NC2, 128), bf))[d] = \
            b.astype(bf).reshape(NC2, 128)
    shared["sel16"] = np.kron(np.eye(NC1), np.ones((1, 32))).astype(bf)
    shared["sel8"] = np.kron(np.eye(NC2), np.ones((1, 32))).astype(bf)
    shared["linWT"] = np.ascontiguousarray(g["lin_w"].T.astype(bf))
    shared["lin_b"] = g["lin_b"].astype(bf)[None, :]
    shared["transitions"] = g["transitions"].astype(np.float32)
    shared["start_trans"] = g["start_trans"].astype(np.float32)[:, None]
    shared["end_trans"] = g["end_trans"].astype(np.float32)[:, None]
    sel = (np.arange(128)[:, None] % 8 == np.arange(8)[None, :]).astype(np.float32)
    shared["sel"] = sel
    mask = np.ones((128, 1), np.float32)
    mask[120:128] = 0.0
    shared["mask_last"] = mask

    in_maps = []
    for cix in range(8):
        b0 = cix * BPC
        m = dict(shared)
        sl = slice(b0, b0 + BPC)
        seq_c = seq[sl]  # [8, T]
        tags_c = tags[sl]
        tok_seq = seq_c.T.reshape(NTOK).astype(np.int32)  # token-major (t, b)
        tok_tags = tags_c.T.reshape(NTOK).astype(np.int64)
        m["seq_tok"] = tok_seq[:, None]
        m["idx_emit"] = (np.arange(NTOK, dtype=np.int64) * TAGS + tok_tags).astype(np.int32)[:, None]
        nxt = np.concatenate([tok_tags[BPC:], np.zeros(BPC, np.int64)])
        it = tok_tags * TAGS + nxt
        it[-BPC:] = 0
        m["idx_trans"] = it.astype(np.int32)[:, None]
        m["idx_start"] = tags_c[:, 0].astype(np.int32)[:, None]
        m["idx_end"] = tags_c[:, T - 1].astype(np.int32)[:, None]

        # state layouts: [128p, (block, batch)]
        def pack_state(arr, nb):  # arr [8, nb*128] -> [2? no: [128, nb*8]
            return np.ascontiguousarray(
                arr.reshape(BPC, nb, 128).transpose(2, 1, 0).reshape(128, nb * BPC))

        m["h0l1"] = np.stack([pack_state(g["h0"][d, sl], KB1) for d in range(2)]).astype(bf)
        m["c0l1"] = np.stack([pack_state(g["c0"][d, sl], KB1) for d in range(2)]).astype(np.float32)
        m["h0l2"] = np.stack([pack_state(g["h1"][d, sl], KB2) for d in range(2)]).astype(bf)
        m["c0l2"] = np.stack([pack_state(g["c1"][d, sl], KB2) for d in range(2)]).astype(np.float32)
        in_maps.append(m)
    return in_maps


def kernel(**inputs) -> np.ndarray:
    import time
    from concourse.bass_utils import run_bass_kernel_spmd

    if "nc" not in _CACHE:
        _CACHE["nc"] = _build_program()
    nc = _CACHE["nc"]

    in_maps = _prep_inputs(inputs)
    res = None
    for attempt in range(3):
        try:
            res = run_bass_kernel_spmd(nc, in_maps, core_ids=list(range(8)))
            break
        except Exception:
            # transient NRT_EXEC_UNIT_UNRECOVERABLE after wedged runs —
            # observed to recover after ~60s
            if attempt == 2:
                raise
            time.sleep(60)
    loss = np.float64(0.0)
    for r in res.results:
        o = r["out"].astype(np.float64)  # [2, 8]: joint, logz
        loss += np.sum(o[0] - o[1])
    return np.float32(-loss)


# revision 41
# speedup vs baseline: 2.9905x; 1.1009x over previous
"""BiLSTM-CRF forward loss on 8 Trainium2 cores (batch-parallel SPMD).

Layout/sharding summary (per core, b=8 examples of B=64):
- embedding gather -> x^T (PE transposes) kept in SBUF
- LSTM scans run W-stationary: gates live TRANSPOSED in PSUM as
  [128p = gate-unit-in-chunk, cols = (chunk, step-in-group, batch)].
  Per 4-step group: one bias selector matmul + input-projection matmuls
  (4 steps wide); per step: tiny [128,8]-output recurrent matmuls.
  Elementwise cell update on full-128-partition tiles; h written
  straight into a transposed SBUF stash (no per-step transposes).
- L2 scan reads the L1 stash directly (no DRAM round trip).
- linear -> logitsT [48, T*8] and logits [T*8, 48]
- CRF forward pass in exp-domain: alpha_t = (expT^T @ alpha) * exp(emit_t)
- gold path score via indirect-DMA gathers + selector matmuls
Outputs per core: [2, 8] fp32 (row0 joint, row1 logZ). Host sums
-(joint - logZ) over all 64 examples.
"""

import numpy as np

B, T, VOCAB, EMBED, HID, TAGS = 64, 512, 30000, 512, 1024, 48
H1, H2 = HID // 2, HID // 4  # 512, 256
BPC = B // 8  # batch per core = 8
NTOK = T * BPC  # 4096 tokens per core
LN48 = float(np.log(48.0))
NC1, KB1 = 16, 4  # L1 gate chunks (2048/128), hidden blocks (512/128)
NC2, KB2 = 8, 2   # L2 gate chunks (1024/128), hidden blocks (256/128)
KI2 = 8           # L2 input blocks (1024/128)

_CACHE = {}


def _build_program():
    import concourse.bass as bass
    import concourse.tile as tile
    import concourse.mybir as mybir
    from concourse.vector_clock import ScopedClock, VectorClock
    from concourse.masks import make_identity

    def _patched_drain_and_barrier(self, tick_clock, wait_clock):
        # This container's walrus rejects >2 sem waits on one CTRL
        # instruction; split the kernel-tail drain waits into per-proc
        # NOP waits on the same (in-order) SP queue.
        vc = tick_clock.global_clock
        n = len(vc)
        for p in range(n):
            t = vc[p]
            if t > 0:
                vec = [0] * n
                vec[p] = t
                nop = self.nc.sync.nop()
                wait_clock.add_sem_waits(nop.ins, ScopedClock({None: VectorClock(vec)}))
        self.nc.sync.drain()
        self.nc.all_engine_barrier()
        popped = self.nc._tile_sem_poison_stack.pop()
        assert popped is self._sem_poison
        self.nc.clear_and_free_semaphores(list(self.sems.allocated().values()))
        self.nc.all_engine_barrier()

    tile.TileContext._drain_and_barrier = _patched_drain_and_barrier

    f32 = mybir.dt.float32
    bf16 = mybir.dt.bfloat16
    i32 = mybir.dt.int32
    ACT = mybir.ActivationFunctionType
    ADD = mybir.AluOpType.add
    MULT = mybir.AluOpType.mult

    nc = bass.Bass()
    PH = int(__import__("os").environ.get("KPHASES", "99"))
    PROBE = int(__import__("os").environ.get("KPROBE", "0"))

    # ---------------- inputs ----------------
    def din(name, shape, dt=f32):
        return nc.dram_tensor(name, shape, dt, kind="ExternalInput")

    embed_bf = din("embed_bf", [VOCAB, EMBED], bf16)
    seq_tok = din("seq_tok", [NTOK, 1], i32)
    idx_emit = din("idx_emit", [NTOK, 1], i32)
    idx_trans = din("idx_trans", [NTOK, 1], i32)
    idx_start = din("idx_start", [BPC, 1], i32)
    idx_end = din("idx_end", [BPC, 1], i32)
    w1x_d = din("w1x", [2, EMBED, 4 * H1], bf16)   # W_ih1^T, perm'd gate cols
    w1h_d = din("w1h", [2, H1, 4 * H1], bf16)      # W_hh1^T
    b1_d = din("b1", [2, NC1, 128], bf16)          # bias rows per chunk
    w2x_d = din("w2x", [2, 2 * H1, 4 * H2], bf16)
    w2h_d = din("w2h", [2, H2, 4 * H2], bf16)
    b2_d = din("b2", [2, NC2, 128], bf16)
    sel16_d = din("sel16", [NC1, 512], bf16)       # chunk-selector, 32-wide
    sel8_d = din("sel8", [NC2, 256], bf16)
    linWT = din("linWT", [H2 * 2, TAGS], bf16)
    lin_b = din("lin_b", [1, TAGS], bf16)
    h0l1_d = din("h0l1", [2, 128, 8 * KB1], bf16)  # (p, b, j)
    c0l1_d = din("c0l1", [2, 128, 8 * KB1])
    h0l2_d = din("h0l2", [2, 128, 8 * KB2], bf16)
    c0l2_d = din("c0l2", [2, 128, 8 * KB2])
    trans_d = din("transitions", [TAGS, TAGS])
    start_d = din("start_trans", [TAGS, 1])
    end_d = din("end_trans", [TAGS, 1])
    sel_d = din("sel", [128, BPC])  # sel[p, b] = (p % 8 == b), fp32
    mask_d = din("mask_last", [128, 1])  # 1.0 except rows 120..127 -> 0.0

    out_d = nc.dram_tensor("out", [2, BPC], f32, kind="ExternalOutput")

    NM = NTOK // 128  # 32 token chunks

    with tile.TileContext(nc) as tc:
        with tc.tile_pool(name="dram", bufs=1, space="DRAM") as dpool, \
             tc.tile_pool(name="const", bufs=1) as cpool, \
             tc.tile_pool(name="persist", bufs=1) as ppool:

            logits_t = dpool.tile([NTOK, TAGS], f32)

            ones_bf = cpool.tile([1, 512], bf16)
            nc.gpsimd.memset(ones_bf[:], 1.0)
            ones_f = cpool.tile([128, 1], f32)
            nc.gpsimd.memset(ones_f[:], 1.0)
            id128 = cpool.tile([128, 128], bf16)
            make_identity(nc, id128[:])
            sel_sb = cpool.tile([128, BPC], f32)
            nc.sync.dma_start(sel_sb[:], sel_d[:])
            mask_sb = cpool.tile([128, 1], f32)
            nc.sync.dma_start(mask_sb[:], mask_d[:])

            # persistent state across phases
            stash1 = [ppool.tile([128, KB1 * NTOK], bf16, tag=f"st1_{d}",
                                 name=f"st1_{d}") for d in range(2)]
            stash2 = [ppool.tile([128, KB2 * NTOK], bf16, tag=f"st2_{d}",
                                 name=f"st2_{d}") for d in range(2)]


            # ============ P1 (embed gather -> xT sbuf) + P3 (L1 scan) ======
            with tc.tile_pool(name="w13", bufs=1) as wp, \
                 tc.tile_pool(name="p1", bufs=2) as sp1, \
                 tc.tile_pool(name="p1p", bufs=2, space="PSUM") as ps1, \
                 tc.tile_pool(name="st3", bufs=1) as stp, \
                 tc.tile_pool(name="p3", bufs=6) as sp3, \
                 tc.tile_pool(name="p3g", bufs=3, space="PSUM") as psg:

                xT = [wp.tile([128, NTOK], bf16, tag=f"xT{k}", name=f"xT{k}")
                      for k in range(KB1)]
                # interleave low/high chunks so both scan directions start early
                ms = []
                for a in range(NM // 2):
                    ms += [a, NM - 1 - a]
                for m in (ms if PH >= 1 else []):
                    idx = sp1.tile([128, 1], i32, tag="idx")
                    nc.sync.dma_start(idx[:], seq_tok[128 * m:128 * (m + 1), :])
                    xg = sp1.tile([128, EMBED], bf16, tag="xg")
                    nc.gpsimd.indirect_dma_start(
                        out=xg[:], out_offset=None, in_=embed_bf[:],
                        in_offset=bass.IndirectOffsetOnAxis(ap=idx[:, :1], axis=0))
                    for e in range(EMBED // 128):
                        pt = ps1.tile([128, 128], bf16, space="PSUM", tag="pt")
                        nc.tensor.transpose(out=pt[:], in_=xg[:, 128 * e:128 * (e + 1)],
                                            identity=id128[:])
                        if e % 2 == 0:
                            nc.vector.tensor_copy(xT[e][:, 128 * m:128 * (m + 1)], pt[:])
                        else:
                            nc.scalar.copy(xT[e][:, 128 * m:128 * (m + 1)], pt[:])

                # ---- L1 weights ----
                w1x = {}
                w1h = {}
                for d in range(2):
                    for k in range(KB1):
                        w = wp.tile([128, 4 * H1], bf16, tag=f"w1x_{d}_{k}")
                        nc.sync.dma_start(w[:], w1x_d[d, 128 * k:128 * (k + 1), :])
                        w1x[(d, k)] = w
                        w = wp.tile([128, 4 * H1], bf16, tag=f"w1h_{d}_{k}")
                        nc.sync.dma_start(w[:], w1h_d[d, 128 * k:128 * (k + 1), :])
                        w1h[(d, k)] = w
                b1t = []
                for d in range(2):
                    b = wp.tile([NC1, 128], bf16, tag=f"b1_{d}")
                    nc.sync.dma_start(b[:], b1_d[d])
                    b1t.append(b)
                sel16 = wp.tile([NC1, 512], bf16, tag="sel16")
                nc.sync.dma_start(sel16[:], sel16_d[:])

                cS = {}
                h0t1 = []
                for d in range(2):
                    for par in range(2):
                        c = stp.tile([128, 8 * KB1], f32, tag=f"c1_{d}_{par}")
                        cS[(d, par)] = c
                    nc.sync.dma_start(cS[(d, 0)][:], c0l1_d[d])
                    h0 = stp.tile([128, 8 * KB1], bf16, tag=f"h01_{d}")
                    nc.sync.dma_start(h0[:], h0l1_d[d])
                    h0t1.append(h0)

                # ---- L1 scan: 4-step groups ----
                for g in range(T // 4 if PH >= 3 else 0):
                    s0 = 4 * g
                    pgs = []
                    for d in range(2):
                        pg = psg.tile([128, 512], f32, space="PSUM", tag=f"pg{d}")
                        pgs.append(pg)
                        tb = s0 if d == 0 else T - 4 - s0  # lowest t in group
                        nc.tensor.matmul(pg[:], lhsT=b1t[d][:], rhs=sel16[:],
                                         start=True, stop=False,
                                         skip_group_check=True)
                        for c in range(NC1 if PROBE != 1 else 0):
                            for k in range(KB1):
                                nc.tensor.matmul(
                                    pg[:, 32 * c:32 * c + 32],
                                    lhsT=w1x[(d, k)][:, 128 * c:128 * (c + 1)],
                                    rhs=xT[k][:, 8 * tb:8 * tb + 32],
                                    start=False, stop=False,
                                    skip_group_check=True)
                    for i in range(4):
                        s = s0 + i
                        # stage-interleaved across dirs so the Act FIFO never
                        # serializes one dir's chain behind the other's
                        tps = [i, 3 - i]
                        ts = [s, T - 1 - s]
                        ves = [nc.vector, nc.gpsimd]
                        pgvs = [pgs[d][:].rearrange("p (c q) -> p c q", q=32)
                                for d in range(2)]

                        def hmm1(d, c):
                            tp = tps[d]
                            for k in range(KB1):
                                if s == 0:
                                    rhs = h0t1[d][:, 8 * k:8 * k + 8]
                                else:
                                    tprev = (s - 1) if d == 0 else (T - s)
                                    o = k * NTOK + 8 * tprev
                                    rhs = stash1[d][:, o:o + 8]
                                nc.tensor.matmul(
                                    pgs[d][:, 32 * c + 8 * tp:32 * c + 8 * tp + 8],
                                    lhsT=w1h[(d, k)][:, 128 * c:128 * (c + 1)],
                                    rhs=rhs, start=False, stop=(k == KB1 - 1),
                                    skip_group_check=True)
                        if PROBE == 2:
                            # floor probe: h-MMs + one copy to stash only
                            for d in range(2):
                                for c in range(4):
                                    hmm1(d, c)
                            for d in range(2):
                                hout = stash1[d][:].rearrange(
                                    "p (b n) -> p b n",
                                    b=KB1)[:, :, 8 * ts[d]:8 * ts[d] + 8]
                                nc.scalar.copy(
                                    hout, pgvs[d][:, 0:4,
                                                  8 * tps[d]:8 * tps[d] + 8])
                            continue
                        # gate-type chunk order: (g, i, f, o); o-MMs last,
                        # sigmoid(o) off the critical chain.
                        for d in range(2):
                            for c in range(12):
                                hmm1(d, c)
                        # tanh(x) = 2*sigmoid(2x) - 1; the 2x for the g gate
                        # is folded into W/bias host-side: one sigmoid for
                        # g, i, f.
                        sgs, sos, scs = [], [], []
                        for d in range(2):
                            sg = sp3.tile([128, 96], bf16, tag=f"sg{d}")
                            nc.scalar.activation(
                                sg[:].rearrange("p (c q) -> p c q", q=8),
                                pgvs[d][:, 0:12, 8 * tps[d]:8 * tps[d] + 8],
                                ACT.Sigmoid)
                            sgs.append(sg)
                        for d in range(2):
                            for c in range(12, NC1):
                                hmm1(d, c)
                        for d in range(2):
                            ve, sg = ves[d], sgs[d]
                            gt = sp3.tile([128, 32], bf16, tag=f"gt{d}")
                            ve.tensor_scalar(gt[:], sg[:, 0:32], 2.0, -1.0,
                                             MULT, ADD)
                            c_old = cS[(d, s % 2)]
                            c_new = cS[(d, (s + 1) % 2)]
                            t1 = sp3.tile([128, 32], f32, tag=f"t1{d}")
                            ve.tensor_tensor(out=t1[:], in0=sg[:, 64:96],
                                             in1=c_old[:], op=MULT)
                            t2 = sp3.tile([128, 32], f32, tag=f"t2{d}")
                            ve.tensor_tensor(out=t2[:], in0=sg[:, 32:64],
                                             in1=gt[:], op=MULT)
                            ve.tensor_tensor(out=c_new[:], in0=t1[:],
                                             in1=t2[:], op=ADD)
                        for d in range(2):
                            sc = sp3.tile([128, 32], bf16, tag=f"sc{d}")
                            nc.scalar.activation(sc[:], cS[(d, (s + 1) % 2)][:],
                                                 ACT.Tanh)
                            scs.append(sc)
                        for d in range(2):
                            so = sp3.tile([128, 32], bf16, tag=f"so{d}")
                            nc.scalar.activation(
                                so[:].rearrange("p (c q) -> p c q", q=8),
                                pgvs[d][:, 12:16, 8 * tps[d]:8 * tps[d] + 8],
                                ACT.Sigmoid)
                            sos.append(so)
                        for d in range(2):
                            ve = ves[d]
                            hout = stash1[d][:].rearrange(
                                "p (b n) -> p b n",
                                b=KB1)[:, :, 8 * ts[d]:8 * ts[d] + 8]
                            ve.tensor_tensor(
                                out=hout,
                                in0=sos[d][:].rearrange("p (b q) -> p b q", q=8),
                                in1=scs[d][:].rearrange("p (b q) -> p b q", q=8),
                                op=MULT)

            # ================= P5: L2 scan ================================
            with tc.tile_pool(name="w5", bufs=1) as wp, \
                 tc.tile_pool(name="st5", bufs=1) as stp, \
                 tc.tile_pool(name="p5", bufs=4) as sp5, \
                 tc.tile_pool(name="p5g", bufs=2, space="PSUM") as psg:
                w2x = {}
                w2h = {}
                for d in range(2):
                    for k in range(KI2):
                        w = wp.tile([128, 4 * H2], bf16, tag=f"w2x_{d}_{k}")
                        nc.sync.dma_start(w[:], w2x_d[d, 128 * k:128 * (k + 1), :])
                        w2x[(d, k)] = w
                    for k in range(KB2):
                        w = wp.tile([128, 4 * H2], bf16, tag=f"w2h_{d}_{k}")
                        nc.sync.dma_start(w[:], w2h_d[d, 128 * k:128 * (k + 1), :])
                        w2h[(d, k)] = w
                b2t = []
                for d in range(2):
                    b = wp.tile([NC2, 128], bf16, tag=f"b2_{d}")
                    nc.sync.dma_start(b[:], b2_d[d])
                    b2t.append(b)
                sel8 = wp.tile([NC2, 256], bf16, tag="sel8")
                nc.sync.dma_start(sel8[:], sel8_d[:])

                c2 = {}
                h0t2 = []
                for d in range(2):
                    for par in range(2):
                        c = stp.tile([128, 8 * KB2], f32, tag=f"c2_{d}_{par}")
                        c2[(d, par)] = c
                    nc.sync.dma_start(c2[(d, 0)][:], c0l2_d[d])
                    h0 = stp.tile([128, 8 * KB2], bf16, tag=f"h02_{d}")
                    nc.sync.dma_start(h0[:], h0l2_d[d])
                    h0t2.append(h0)

                for g in range(T // 4 if PH >= 5 else 0):
                    s0 = 4 * g
                    pgs5 = []
                    for d in range(2):
                        pg = psg.tile([128, 256], f32, space="PSUM", tag=f"pg5{d}")
                        pgs5.append(pg)
                        tb = s0 if d == 0 else T - 4 - s0
                        nc.tensor.matmul(pg[:], lhsT=b2t[d][:],
                                         rhs=sel8[:], start=True, stop=False,
                                         skip_group_check=True)
                        for c in range(NC2):
                            for k in range(KI2):
                                src = stash1[k // KB1][
                                    :, (k % KB1) * NTOK + 8 * tb:
                                    (k % KB1) * NTOK + 8 * tb + 32]
                                nc.tensor.matmul(
                                    pg[:, 32 * c:32 * c + 32],
                                    lhsT=w2x[(d, k)][:, 128 * c:128 * (c + 1)],
                                    rhs=src, start=False, stop=False,
                                    skip_group_check=True)
                    for i in range(4):
                        s = s0 + i
                        tps = [i, 3 - i]
                        ts = [s, T - 1 - s]
                        ves = [nc.vector, nc.gpsimd]
                        pgvs5 = [pgs5[d][:].rearrange("p (c q) -> p c q", q=32)
                                 for d in range(2)]

                        def hmm2(d, c):
                            tp = tps[d]
                            for k in range(KB2):
                                if s == 0:
                                    rhs = h0t2[d][:, 8 * k:8 * k + 8]
                                else:
                                    tprev = (s - 1) if d == 0 else (T - s)
                                    o = k * NTOK + 8 * tprev
                                    rhs = stash2[d][:, o:o + 8]
                                nc.tensor.matmul(
                                    pgs5[d][:, 32 * c + 8 * tp:
                                            32 * c + 8 * tp + 8],
                                    lhsT=w2h[(d, k)][:, 128 * c:128 * (c + 1)],
                                    rhs=rhs, start=False, stop=(k == KB2 - 1),
                                    skip_group_check=True)
                        for d in range(2):
                            for c in range(NC2):
                                hmm2(d, c)
                        sgs, sos, scs = [], [], []
                        for d in range(2):
                            sg = sp5.tile([128, 64], bf16, tag=f"sg5{d}")
                            nc.scalar.activation(
                                sg[:].rearrange("p (c q) -> p c q", q=8),
                                pgvs5[d][:, 0:8, 8 * tps[d]:8 * tps[d] + 8],
                                ACT.Sigmoid)
                            sgs.append(sg)
                            sos.append(sg[:, 48:64])
                        for d in range(2):
                            ve, sg = ves[d], sgs[d]
                            gt = sp5.tile([128, 16], bf16, tag=f"gt5{d}")
                            ve.tensor_scalar(gt[:], sg[:, 0:16], 2.0, -1.0,
                                             MULT, ADD)
                            c_old = c2[(d, s % 2)]
                            c_new = c2[(d, (s + 1) % 2)]
                            t1 = sp5.tile([128, 16], f32, tag=f"t15{d}")
                            ve.tensor_tensor(out=t1[:], in0=sg[:, 32:48],
                                             in1=c_old[:], op=MULT)
                            t2 = sp5.tile([128, 16], f32, tag=f"t25{d}")
                            ve.tensor_tensor(out=t2[:], in0=sg[:, 16:32],
                                             in1=gt[:], op=MULT)
                            ve.tensor_tensor(out=c_new[:], in0=t1[:],
                                             in1=t2[:], op=ADD)
                        for d in range(2):
                            sc = sp5.tile([128, 16], bf16, tag=f"sc5{d}")
                            nc.scalar.activation(sc[:], c2[(d, (s + 1) % 2)][:],
                                                 ACT.Tanh)
                            scs.append(sc)
                        for d in range(2):
                            ve = ves[d]
                            hout = stash2[d][:].rearrange(
                                "p (b n) -> p b n",
                                b=KB2)[:, :, 8 * ts[d]:8 * ts[d] + 8]
                            ve.tensor_tensor(
                                out=hout,
                                in0=sos[d].rearrange("p (b q) -> p b q", q=8),
                                in1=scs[d][:].rearrange("p (b q) -> p b q", q=8),
                                op=MULT)

            # ================= P6: linear -> Esb + logits ==================
            epool_cm = tc.tile_pool(name="e67", bufs=1)
            epool = epool_cm.__enter__()
            Esb = epool.tile([TAGS, NTOK], bf16, tag="Esb")
            with tc.tile_pool(name="w6", bufs=1) as wp, \
                 tc.tile_pool(name="p6", bufs=3) as sp, \
                 tc.tile_pool(name="p6p", bufs=4, space="PSUM") as psp:
                lw = [wp.tile([128, TAGS], bf16, tag=f"lw{q}", name=f"lw{q}")
                      for q in range(4)]
                for q in range(4):
                    nc.sync.dma_start(lw[q][:], linWT[128 * q:128 * (q + 1), :])
                lb = wp.tile([1, TAGS], bf16, tag="lb")
                nc.sync.dma_start(lb[:], lin_b[:])
                # logitsT [48, NTOK] (+ exp) straight from the h2 stash
                for n in range(NTOK // 512 if PH >= 6 else 0):
                    pg = psp.tile([TAGS, 512], f32, space="PSUM", tag="pl")
                    for q in range(4):
                        d, bq = q // 2, q % 2
                        nc.tensor.matmul(
                            pg[:], lhsT=lw[q][:],
                            rhs=stash2[d][:, bq * NTOK + 512 * n:
                                          bq * NTOK + 512 * (n + 1)],
                            start=(q == 0), stop=False)
                    nc.tensor.matmul(pg[:], lhsT=lb[0:1, :], rhs=ones_bf[0:1, :],
                                     start=False, stop=True)
                    nc.scalar.activation(Esb[:, 512 * n:512 * (n + 1)],
                                         pg[:], ACT.Exp)
                # logits [NTOK, 48] to DRAM for gathers
                for m in range(NM if PH >= 6 else 0):
                    pg = psp.tile([128, TAGS], f32, space="PSUM", tag="pl2")
                    for q in range(4):
                        d, bq = q // 2, q % 2
                        nc.tensor.matmul(
                            pg[:], lhsT=stash2[d][:, bq * NTOK + 128 * m:
                                                  bq * NTOK + 128 * (m + 1)],
                            rhs=lw[q][:], start=(q == 0), stop=False)
                    nc.tensor.matmul(pg[:], lhsT=ones_bf[0:1, 0:128], rhs=lb[0:1, :],
                                     start=False, stop=True)
                    ls = sp.tile([128, TAGS], f32, tag="ls")
                    nc.scalar.copy(ls[:], pg[:])
                    nc.sync.dma_start(logits_t[128 * m:128 * (m + 1), :], ls[:])

            # ================= P8: gold path score =========================
            with tc.tile_pool(name="p8", bufs=3) as sp, \
                 tc.tile_pool(name="p8a", bufs=1) as aw, \
                 tc.tile_pool(name="p8p", bufs=2, space="PSUM") as psp:
                accW = aw.tile([128, NM], f32, tag="accW")
                lfl = logits_t[:].rearrange("n k -> (n k)").unsqueeze(1)
                tfl = trans_d[:].rearrange("a b -> (a b)").unsqueeze(1)
                for m in range(NM if PH >= 8 else 0):
                    ie = sp.tile([128, 1], i32, tag="ie")
                    nc.sync.dma_start(ie[:], idx_emit[128 * m:128 * (m + 1), :])
                    it = sp.tile([128, 1], i32, tag="it")
                    nc.sync.dma_start(it[:], idx_trans[128 * m:128 * (m + 1), :])
                    ge = sp.tile([128, 1], f32, tag="ge")
                    nc.gpsimd.indirect_dma_start(
                        out=ge[:], out_offset=None, in_=lfl,
                        in_offset=bass.IndirectOffsetOnAxis(ap=ie[:, :1], axis=0))
                    gt = sp.tile([128, 1], f32, tag="gt")
                    nc.gpsimd.indirect_dma_start(
                        out=gt[:], out_offset=None, in_=tfl,
                        in_offset=bass.IndirectOffsetOnAxis(ap=it[:, :1], axis=0))
                    if m == NM - 1:
                        nc.vector.tensor_tensor(out=gt[:], in0=gt[:],
                                                in1=mask_sb[:], op=MULT)
                    nc.vector.tensor_tensor(out=accW[:, m:m + 1], in0=ge[:],
                                            in1=gt[:], op=ADD)
                se16 = sp.tile([16, 1], f32, tag="se16")
                ist = sp.tile([BPC, 1], i32, tag="ist")
                nc.sync.dma_start(ist[:], idx_start[:])
                nc.gpsimd.indirect_dma_start(
                    out=se16[0:8, :], out_offset=None,
                    in_=start_d[:].rearrange("a b -> (a b)").unsqueeze(1),
                    in_offset=bass.IndirectOffsetOnAxis(ap=ist[:, :1], axis=0))
                ien = sp.tile([BPC, 1], i32, tag="ien")
                nc.sync.dma_start(ien[:], idx_end[:])
                nc.gpsimd.indirect_dma_start(
                    out=se16[8:16, :], out_offset=None,
                    in_=end_d[:].rearrange("a b -> (a b)").unsqueeze(1),
                    in_offset=bass.IndirectOffsetOnAxis(ap=ien[:, :1], axis=0))
                if PH < 8:
                    nc.gpsimd.memset(accW[:], 0.0)
                s1 = psp.tile([NM, BPC], f32, space="PSUM", tag="s1")
                nc.tensor.matmul(s1[:], lhsT=accW[:], rhs=sel_sb[:],
                                 start=True, stop=True)
                s1s = sp.tile([NM, BPC], f32, tag="s1s")
                nc.scalar.copy(s1s[:], s1[:])
                s2 = psp.tile([1, BPC], f32, space="PSUM", tag="s2")
                nc.tensor.matmul(s2[:], lhsT=ones_f[0:NM, 0:1], rhs=s1s[:],
                                 start=True, stop=False)
                nc.tensor.matmul(s2[:], lhsT=se16[:], rhs=sel_sb[0:16, :],
                                 start=False, stop=True)
                joint = sp.tile([1, BPC], f32, tag="joint")
                nc.scalar.copy(joint[:], s2[:])
                nc.sync.dma_start(out_d[0:1, :], joint[:])

            # ================= P7: CRF partition (exp domain) ==============
            with tc.tile_pool(name="p7", bufs=1) as sp, \
                 tc.tile_pool(name="p7a", bufs=4) as ap, \
                 tc.tile_pool(name="p7p", bufs=2, space="PSUM") as psp, \
                 tc.tile_pool(name="p7f", bufs=1, space="PSUM") as psf:
                tr = sp.tile([TAGS, TAGS], f32, tag="tr")
                nc.sync.dma_start(tr[:], trans_d[:])
                ETp = sp.tile([TAGS, TAGS], bf16, tag="ETp")
                nln48 = sp.tile([TAGS, 1], f32, tag="nln48")
                nc.gpsimd.memset(nln48[:], -LN48)
                nc.scalar.activation(ETp[:], tr[:], ACT.Exp, bias=nln48[:, 0:1])
                stv = sp.tile([TAGS, 1], f32, tag="stv")
                nc.sync.dma_start(stv[:], start_d[:])
                estart = sp.tile([TAGS, 1], f32, tag="estart")
                nc.scalar.activation(estart[:], stv[:], ACT.Exp)
                env = sp.tile([TAGS, 1], f32, tag="env")
                nc.sync.dma_start(env[:], end_d[:])
                eend = sp.tile([TAGS, 1], bf16, tag="eend")
                nc.scalar.activation(eend[:], env[:], ACT.Exp)

                alpha = ap.tile([TAGS, BPC], bf16, tag="alpha")
                if PH >= 7:
                    nc.vector.tensor_scalar_mul(alpha[:], Esb[:, 0:BPC],
                                                estart[:, 0:1])
                else:
                    nc.gpsimd.memset(alpha[:], 1.0)
                for t in range(1, T if PH >= 7 else 1):
                    pm = psp.tile([TAGS, BPC], f32, space="PSUM", tag="pm")
                    nc.tensor.matmul(pm[:], lhsT=ETp[:], rhs=alpha[:],
                                     start=True, stop=True)
                    alpha = ap.tile([TAGS, BPC], bf16, tag="alpha")
                    nc.vector.tensor_tensor(out=alpha[:], in0=pm[:],
                                            in1=Esb[:, BPC * t:BPC * (t + 1)], op=MULT)
                pf = psf.tile([1, BPC], f32, space="PSUM", tag="pf")
                nc.tensor.matmul(pf[:], lhsT=eend[:], rhs=alpha[:],
                                 start=True, stop=True)
                logz = sp.tile([1, BPC], f32, tag="logz")
                nc.scalar.activation(logz[:], pf[:], ACT.Ln)
                nc.vector.tensor_scalar_add(logz[:], logz[:], float((T - 1) * LN48))
                nc.sync.dma_start(out_d[1:2, :], logz[:])
            epool_cm.__exit__(None, None, None)
    _split_waits(nc, maxw=int(__import__("os").environ.get("KMAXW", "1")))
    return nc


def _split_waits(nc, maxw=2):
    """This container's walrus rejects instructions carrying more than a
    couple of semaphore waits. Hoist extras onto preceding same-engine
    NoOps (engines execute their stream in order, so this preserves the
    happens-before)."""
    import concourse.mybir as mybir
    import bass_rust
    compute_ops = {"Matmult", "Activation", "TensorTensor", "TensorScalar",
                   "TensorCopy", "TensorReduce", "Memset", "Iota",
                   "AffineSelect", "TensorTensorScan", "Select"}
    n_added = 0
    for fn in nc.m.functions:
        for blk in fn.blocks:
            insts = list(blk.instructions)
            out = []
            dirty = False
            for inst in insts:
                mw = 2 if (maxw == 0 and str(inst.opcode) in compute_ops) else max(1, maxw)
                si = inst.sync_info
                if si is not None and len(si.on_wait) > mw:
                    waits = list(si.on_wait)
                    extra, keep = waits[:-mw], waits[-mw:]
                    for i in range(0, len(extra), mw):
                        nop = mybir.InstNoOp(
                            name=f"{inst.name}_hw{i}", ins=[], outs=[])
                        nop.engine = inst.engine
                        nop.sync_info = bass_rust.SyncInfo(
                            on_wait=extra[i:i + mw], on_update=[])
                        out.append(nop)
                        n_added += 1
                    inst.sync_info = bass_rust.SyncInfo(
                        on_wait=keep, on_update=list(si.on_update))
                    dirty = True
                out.append(inst)
            if dirty:
                blk.instructions = out
    return n_added


def _prep_inputs(inputs):
    import ml_dtypes
    bf = ml_dtypes.bfloat16

    g = {k: np.asarray(v) for k, v in inputs.items()}
    seq = g["sequences"].astype(np.int64)
    tags = g["tags"].astype(np.int64)

    # gate-chunk permutation: chunk c = 4*t_idx + b (L1) / 2*t_idx + b (L2),
    # t_idx order (g, i, f, o) from pytorch row order (i, f, g, o)
    def gate_perm(h, nb):
        return np.concatenate([
            np.arange(gs * h + 128 * b, gs * h + 128 * b + 128)
            for gs in [2, 0, 1, 3] for b in range(nb)])

    p1 = gate_perm(H1, KB1)
    p2 = gate_perm(H2, KB2)

    shared = {}
    shared["embed_bf"] = np.ascontiguousarray(g["embed_table"].astype(bf))
    for d, sfx in enumerate(["1f", "1b"]):
        wih = g["w_ih" + sfx][p1].astype(np.float64)  # [2048, 512] perm'd rows
        whh = g["w_hh" + sfx][p1].astype(np.float64)
        b = ((g["b_ih" + sfx] + g["b_hh" + sfx])[p1]).astype(np.float64)
        # tanh(x) = 2*sigmoid(2x)-1: pre-double the g-gate rows (chunks 0..3)
        wih[:4 * 128] *= 2.0
        whh[:4 * 128] *= 2.0
        b[:4 * 128] *= 2.0
        shared.setdefault("w1x", np.zeros((2, EMBED, 4 * H1), bf))[d] = wih.T.astype(bf)
        shared.setdefault("w1h", np.zeros((2, H1, 4 * H1), bf))[d] = whh.T.astype(bf)
        shared.setdefault("b1", np.zeros((2, NC1, 128), bf))[d] = \
            b.astype(bf).reshape(NC1, 128)
    for d, sfx in enumerate(["2f", "2b"]):
        wih = g["w_ih" + sfx][p2].astype(np.float64)
        whh = g["w_hh" + sfx][p2].astype(np.float64)
        b = ((g["b_ih" + sfx] + g["b_hh" + sfx])[p2]).astype(np.float64)
        wih[:2 * 128] *= 2.0
        whh[:2 * 128] *= 2.0
        b[:2 * 128] *= 2.0
        shared.setdefault("w2x", np.zeros((2, HID, 4 * H2), bf))[d] = wih.T.astype(bf)
        shared.setdefault("w2h", np.zeros((2, H2, 4 * H2), bf))[d] = whh.T.astype(bf)
        shared.setdefault("b2", np.zeros((2, NC2, 128), bf))[d] = \
            b.astype(bf).reshape(NC2, 128)
    shared["sel16"] = np.kron(np.eye(NC1), np.ones((1, 32))).astype(bf)
    shared["sel8"] = np.kron(np.eye(NC2), np.ones((1, 32))).astype(bf)
    shared["linWT"] = np.ascontiguousarray(g["lin_w"].T.astype(bf))
    shared["lin_b"] = g["lin_b"].astype(bf)[None, :]
    shared["transitions"] = g["transitions"].astype(np.float32)
    shared["start_trans"] = g["start_trans"].astype(np.float32)[:, None]
    shared["end_trans"] = g["end_trans"].astype(np.float32)[:, None]
    sel = (np.arange(128)[:, None] % 8 == np.arange(8)[None, :]).astype(np.float32)
    shared["sel"] = sel
    mask = np.ones((128, 1), np.float32)
    mask[120:128] = 0.0
    shared["mask_last"] = mask

    in_maps = []
    for cix in range(8):
        b0 = cix * BPC
        m = dict(shared)
        sl = slice(b0, b0 + BPC)
        seq_c = seq[sl]  # [8, T]
        tags_c = tags[sl]
        tok_seq = seq_c.T.reshape(NTOK).astype(np.int32)  # token-major (t, b)
        tok_tags = tags_c.T.reshape(NTOK).astype(np.int64)
        m["seq_tok"] = tok_seq[:, None]
        m["idx_emit"] = (np.arange(NTOK, dtype=np.int64) * TAGS + tok_tags).astype(np.int32)[:, None]
        nxt = np.concatenate([tok_tags[BPC:], np.zeros(BPC, np.int64)])
        it = tok_tags * TAGS + nxt
        it[-BPC:] = 0
        m["idx_trans"] = it.astype(np.int32)[:, None]
        m["idx_start"] = tags_c[:, 0].astype(np.int32)[:, None]
        m["idx_end"] = tags_c[:, T - 1].astype(np.int32)[:, None]

        # state layouts: [128p, (block, batch)]
        def pack_state(arr, nb):  # arr [8, nb*128] -> [2? no: [128, nb*8]
            return np.ascontiguousarray(
                arr.reshape(BPC, nb, 128).transpose(2, 1, 0).reshape(128, nb * BPC))

        m["h0l1"] = np.stack([pack_state(g["h0"][d, sl], KB1) for d in range(2)]).astype(bf)
        m["c0l1"] = np.stack([pack_state(g["c0"][d, sl], KB1) for d in range(2)]).astype(np.float32)
        m["h0l2"] = np.stack([pack_state(g["h1"][d, sl], KB2) for d in range(2)]).astype(bf)
        m["c0l2"] = np.stack([pack_state(g["c1"][d, sl], KB2) for d in range(2)]).astype(np.float32)
        in_maps.append(m)
    return in_maps


def kernel(**inputs) -> np.ndarray:
    import time
    from concourse.bass_utils import run_bass_kernel_spmd

    if "nc" not in _CACHE:
        _CACHE["nc"] = _build_program()
    nc = _CACHE["nc"]

    in_maps = _prep_inputs(inputs)
    res = None
    for attempt in range(3):
        try:
            res = run_bass_kernel_spmd(nc, in_maps, core_ids=list(range(8)))
            break
        except Exception:
            # transient NRT_EXEC_UNIT_UNRECOVERABLE after wedged runs —
            # observed to recover after ~60s
            if attempt == 2:
                raise
            time.sleep(60)
    loss = np.float64(0.0)
    for r in res.results:
        o = r["out"].astype(np.float64)  # [2, 8]: joint, logz
        loss += np.sum(o[0] - o[1])
    return np.float32(-loss)


# revision 54
# speedup vs baseline: 3.0874x; 1.0324x over previous
"""BiLSTM-CRF forward loss on 8 Trainium2 cores (batch-parallel SPMD).

Layout/sharding summary (per core, b=8 examples of B=64):
- embedding gather -> x^T (PE transposes) kept in SBUF
- LSTM scans run W-stationary: gates live TRANSPOSED in PSUM as
  [128p = gate-unit-in-chunk, cols = (chunk, step-in-group, batch)].
  Per 4-step group: one bias selector matmul + input-projection matmuls
  (4 steps wide); per step: tiny [128,8]-output recurrent matmuls.
  Elementwise cell update on full-128-partition tiles; h written
  straight into a transposed SBUF stash (no per-step transposes).
- L2 scan reads the L1 stash directly (no DRAM round trip).
- linear -> logitsT [48, T*8] and logits [T*8, 48]
- CRF forward pass in exp-domain: alpha_t = (expT^T @ alpha) * exp(emit_t)
- gold path score via indirect-DMA gathers + selector matmuls
Outputs per core: [2, 8] fp32 (row0 joint, row1 logZ). Host sums
-(joint - logZ) over all 64 examples.
"""

import numpy as np

B, T, VOCAB, EMBED, HID, TAGS = 64, 512, 30000, 512, 1024, 48
H1, H2 = HID // 2, HID // 4  # 512, 256
BPC = B // 8  # batch per core = 8
NTOK = T * BPC  # 4096 tokens per core
LN48 = float(np.log(48.0))
NC1, KB1 = 16, 4  # L1 gate chunks (2048/128), hidden blocks (512/128)
NC2, KB2 = 8, 2   # L2 gate chunks (1024/128), hidden blocks (256/128)
KI2 = 8           # L2 input blocks (1024/128)

_CACHE = {}


def _build_program():
    import concourse.bass as bass
    import concourse.tile as tile
    import concourse.mybir as mybir
    from concourse.vector_clock import ScopedClock, VectorClock
    from concourse.masks import make_identity

    def _patched_drain_and_barrier(self, tick_clock, wait_clock):
        # This container's walrus rejects >2 sem waits on one CTRL
        # instruction; split the kernel-tail drain waits into per-proc
        # NOP waits on the same (in-order) SP queue.
        vc = tick_clock.global_clock
        n = len(vc)
        for p in range(n):
            t = vc[p]
            if t > 0:
                vec = [0] * n
                vec[p] = t
                nop = self.nc.sync.nop()
                wait_clock.add_sem_waits(nop.ins, ScopedClock({None: VectorClock(vec)}))
        self.nc.sync.drain()
        self.nc.all_engine_barrier()
        popped = self.nc._tile_sem_poison_stack.pop()
        assert popped is self._sem_poison
        self.nc.clear_and_free_semaphores(list(self.sems.allocated().values()))
        self.nc.all_engine_barrier()

    tile.TileContext._drain_and_barrier = _patched_drain_and_barrier

    f32 = mybir.dt.float32
    bf16 = mybir.dt.bfloat16
    i32 = mybir.dt.int32
    f16 = mybir.dt.float16
    ACT = mybir.ActivationFunctionType
    ADD = mybir.AluOpType.add
    MULT = mybir.AluOpType.mult

    nc = bass.Bass()
    PH = int(__import__("os").environ.get("KPHASES", "99"))
    PROBE = int(__import__("os").environ.get("KPROBE", "0"))

    # ---------------- inputs ----------------
    def din(name, shape, dt=f32):
        return nc.dram_tensor(name, shape, dt, kind="ExternalInput")

    embed_bf = din("embed_bf", [VOCAB, EMBED], bf16)
    seq_tok = din("seq_tok", [NTOK, 1], i32)
    idx_emit = din("idx_emit", [NTOK, 1], i32)
    idx_trans = din("idx_trans", [NTOK, 1], i32)
    idx_start = din("idx_start", [BPC, 1], i32)
    idx_end = din("idx_end", [BPC, 1], i32)
    w1x_d = din("w1x", [2, EMBED, 4 * H1], bf16)   # W_ih1^T, perm'd gate cols
    w1h_d = din("w1h", [2, H1, 4 * H1], bf16)      # W_hh1^T
    b1_d = din("b1", [2, NC1, 128], bf16)          # bias rows per chunk
    w2x_d = din("w2x", [2, 2 * H1, 4 * H2], bf16)
    w2h_d = din("w2h", [2, H2, 4 * H2], bf16)
    b2_d = din("b2", [2, NC2, 128], bf16)
    sel16_d = din("sel16", [NC1, 512], bf16)       # chunk-selector, 32-wide
    sel8_d = din("sel8", [NC2, 256], bf16)
    linWT = din("linWT", [H2 * 2, TAGS], bf16)
    lin_b = din("lin_b", [1, TAGS], bf16)
    h0l1_d = din("h0l1", [2, 128, 8 * KB1], bf16)  # (p, b, j)
    c0l1_d = din("c0l1", [2, 128, 8 * KB1], f16)
    h0l2_d = din("h0l2", [2, 128, 8 * KB2], bf16)
    c0l2_d = din("c0l2", [2, 128, 8 * KB2], f16)
    trans_d = din("transitions", [TAGS, TAGS])
    start_d = din("start_trans", [TAGS, 1])
    end_d = din("end_trans", [TAGS, 1])
    sel_d = din("sel", [128, BPC])  # sel[p, b] = (p % 8 == b), fp32
    mask_d = din("mask_last", [128, 1])  # 1.0 except rows 120..127 -> 0.0

    out_d = nc.dram_tensor("out", [2, BPC], f32, kind="ExternalOutput")

    NM = NTOK // 128  # 32 token chunks

    with tile.TileContext(nc) as tc:
        with tc.tile_pool(name="dram", bufs=1, space="DRAM") as dpool, \
             tc.tile_pool(name="const", bufs=1) as cpool, \
             tc.tile_pool(name="persist", bufs=1) as ppool:

            logits_t = dpool.tile([NTOK, TAGS], f32)

            ones_bf = cpool.tile([1, 512], bf16)
            nc.gpsimd.memset(ones_bf[:], 1.0)
            ones_f = cpool.tile([128, 1], f32)
            nc.gpsimd.memset(ones_f[:], 1.0)
            id128 = cpool.tile([128, 128], bf16)
            make_identity(nc, id128[:])
            sel_sb = cpool.tile([128, BPC], f32)
            nc.sync.dma_start(sel_sb[:], sel_d[:])
            mask_sb = cpool.tile([128, 1], f32)
            nc.sync.dma_start(mask_sb[:], mask_d[:])

            # persistent state across phases
            stash1 = [ppool.tile([128, KB1 * NTOK], bf16, tag=f"st1_{d}",
                                 name=f"st1_{d}") for d in range(2)]
            stash2 = [ppool.tile([128, KB2 * NTOK], bf16, tag=f"st2_{d}",
                                 name=f"st2_{d}") for d in range(2)]


            # ============ P1 (embed gather -> xT sbuf) + P3 (L1 scan) ======
            with tc.tile_pool(name="w13", bufs=1) as wp, \
                 tc.tile_pool(name="p1", bufs=2) as sp1, \
                 tc.tile_pool(name="p1p", bufs=2, space="PSUM") as ps1, \
                 tc.tile_pool(name="st3", bufs=1) as stp, \
                 tc.tile_pool(name="p3", bufs=6) as sp3, \
                 tc.tile_pool(name="p3g", bufs=3, space="PSUM") as psg:

                xT = [wp.tile([128, NTOK], bf16, tag=f"xT{k}", name=f"xT{k}")
                      for k in range(KB1)]
                # interleave low/high chunks so both scan directions start early
                ms = []
                for a in range(NM // 2):
                    ms += [a, NM - 1 - a]
                for m in (ms if PH >= 1 else []):
                    idx = sp1.tile([128, 1], i32, tag="idx")
                    nc.sync.dma_start(idx[:], seq_tok[128 * m:128 * (m + 1), :])
                    xg = sp1.tile([128, EMBED], bf16, tag="xg")
                    nc.gpsimd.indirect_dma_start(
                        out=xg[:], out_offset=None, in_=embed_bf[:],
                        in_offset=bass.IndirectOffsetOnAxis(ap=idx[:, :1], axis=0))
                    for e in range(EMBED // 128):
                        pt = ps1.tile([128, 128], bf16, space="PSUM", tag="pt")
                        nc.tensor.transpose(out=pt[:], in_=xg[:, 128 * e:128 * (e + 1)],
                                            identity=id128[:])
                        if e % 2 == 0:
                            nc.vector.tensor_copy(xT[e][:, 128 * m:128 * (m + 1)], pt[:])
                        else:
                            nc.scalar.copy(xT[e][:, 128 * m:128 * (m + 1)], pt[:])

                # ---- L1 weights ----
                w1x = {}
                w1h = {}
                for d in range(2):
                    for k in range(KB1):
                        w = wp.tile([128, 4 * H1], bf16, tag=f"w1x_{d}_{k}")
                        nc.sync.dma_start(w[:], w1x_d[d, 128 * k:128 * (k + 1), :])
                        w1x[(d, k)] = w
                        w = wp.tile([128, 4 * H1], bf16, tag=f"w1h_{d}_{k}")
                        nc.sync.dma_start(w[:], w1h_d[d, 128 * k:128 * (k + 1), :])
                        w1h[(d, k)] = w
                b1t = []
                for d in range(2):
                    b = wp.tile([NC1, 128], bf16, tag=f"b1_{d}")
                    nc.sync.dma_start(b[:], b1_d[d])
                    b1t.append(b)
                sel16 = wp.tile([NC1, 512], bf16, tag="sel16")
                nc.sync.dma_start(sel16[:], sel16_d[:])

                cS = {}
                h0t1 = []
                for d in range(2):
                    for par in range(2):
                        c = stp.tile([128, 8 * KB1], f16, tag=f"c1_{d}_{par}")
                        cS[(d, par)] = c
                    nc.sync.dma_start(cS[(d, 0)][:], c0l1_d[d])
                    h0 = stp.tile([128, 8 * KB1], bf16, tag=f"h01_{d}")
                    nc.sync.dma_start(h0[:], h0l1_d[d])
                    h0t1.append(h0)

                # ---- L1 scan: 4-step groups ----
                for g in range(T // 4 if PH >= 3 else 0):
                    s0 = 4 * g
                    pgs = []
                    for d in range(2):
                        pg = psg.tile([128, 512], f32, space="PSUM", tag=f"pg{d}")
                        pgs.append(pg)
                        tb = s0 if d == 0 else T - 4 - s0  # lowest t in group
                        nc.tensor.matmul(pg[:], lhsT=b1t[d][:], rhs=sel16[:],
                                         start=True, stop=False,
                                         skip_group_check=True)
                        for c in range(NC1 if PROBE != 1 else 0):
                            for k in range(KB1):
                                nc.tensor.matmul(
                                    pg[:, 32 * c:32 * c + 32],
                                    lhsT=w1x[(d, k)][:, 128 * c:128 * (c + 1)],
                                    rhs=xT[k][:, 8 * tb:8 * tb + 32],
                                    start=False, stop=False,
                                    skip_group_check=True)
                    for i in range(4):
                        s = s0 + i
                        # stage-interleaved across dirs so the Act FIFO never
                        # serializes one dir's chain behind the other's
                        tps = [i, 3 - i]
                        ts = [s, T - 1 - s]
                        ves = [nc.vector, nc.gpsimd]
                        pgvs = [pgs[d][:].rearrange("p (c q) -> p c q", q=32)
                                for d in range(2)]

                        def hmm1(d, c):
                            tp = tps[d]
                            for k in range(KB1):
                                if s == 0:
                                    rhs = h0t1[d][:, 8 * k:8 * k + 8]
                                else:
                                    tprev = (s - 1) if d == 0 else (T - s)
                                    o = k * NTOK + 8 * tprev
                                    rhs = stash1[d][:, o:o + 8]
                                nc.tensor.matmul(
                                    pgs[d][:, 32 * c + 8 * tp:32 * c + 8 * tp + 8],
                                    lhsT=w1h[(d, k)][:, 128 * c:128 * (c + 1)],
                                    rhs=rhs, start=False, stop=(k == KB1 - 1),
                                    skip_group_check=True)
                        if PROBE == 2:
                            # floor probe: h-MMs + one copy to stash only
                            for d in range(2):
                                for c in range(4):
                                    hmm1(d, c)
                            for d in range(2):
                                hout = stash1[d][:].rearrange(
                                    "p (b n) -> p b n",
                                    b=KB1)[:, :, 8 * ts[d]:8 * ts[d] + 8]
                                nc.scalar.copy(
                                    hout, pgvs[d][:, 0:4,
                                                  8 * tps[d]:8 * tps[d] + 8])
                            continue
                        # gate-type chunk order: (g, i, f, o); o-MMs last,
                        # sigmoid(o) off the critical chain.
                        for d in range(2):
                            for c in range(12):
                                hmm1(d, c)
                        # tanh(x) = 2*sigmoid(2x) - 1; the 2x for the g gate
                        # is folded into W/bias host-side: one sigmoid for
                        # g, i, f.
                        sgs, sos, scs = [], [], []
                        for d in range(2):
                            sg = sp3.tile([128, 96], bf16, tag=f"sg{d}")
                            nc.scalar.activation(
                                sg[:].rearrange("p (c q) -> p c q", q=8),
                                pgvs[d][:, 0:12, 8 * tps[d]:8 * tps[d] + 8],
                                ACT.Sigmoid)
                            sgs.append(sg)
                        for d in range(2):
                            for c in range(12, NC1):
                                hmm1(d, c)
                        for d in range(2):
                            so = sp3.tile([128, 32], bf16, tag=f"so{d}")
                            nc.scalar.activation(
                                so[:].rearrange("p (c q) -> p c q", q=8),
                                pgvs[d][:, 12:16, 8 * tps[d]:8 * tps[d] + 8],
                                ACT.Sigmoid)
                            sos.append(so)
                        t1s = []
                        for d in range(2):
                            sg = sgs[d]
                            t1 = sp3.tile([128, 32], f16, tag=f"t1{d}")
                            nc.vector.tensor_tensor(out=t1[:], in0=sg[:, 64:96],
                                                    in1=cS[(d, s % 2)][:],
                                                    op=MULT)
                            t1s.append(t1)
                        for d in range(2):
                            sg = sgs[d]
                            gt = sp3.tile([128, 32], bf16, tag=f"gt{d}")
                            nc.vector.tensor_scalar(gt[:], sg[:, 0:32], 2.0,
                                                    -1.0, MULT, ADD)
                            t2 = sp3.tile([128, 32], f16, tag=f"t2{d}")
                            nc.vector.tensor_tensor(out=t2[:], in0=sg[:, 32:64],
                                                    in1=gt[:], op=MULT)
                            nc.vector.tensor_tensor(out=cS[(d, (s + 1) % 2)][:],
                                                    in0=t1s[d][:], in1=t2[:],
                                                    op=ADD)
                        for d in range(2):
                            sc = sp3.tile([128, 32], bf16, tag=f"sc{d}")
                            nc.scalar.activation(sc[:], cS[(d, (s + 1) % 2)][:],
                                                 ACT.Tanh)
                            scs.append(sc)
                        for d in range(2):
                            hout = stash1[d][:].rearrange(
                                "p (b n) -> p b n",
                                b=KB1)[:, :, 8 * ts[d]:8 * ts[d] + 8]
                            nc.vector.tensor_tensor(
                                out=hout,
                                in0=sos[d][:].rearrange("p (b q) -> p b q", q=8),
                                in1=scs[d][:].rearrange("p (b q) -> p b q", q=8),
                                op=MULT)

            # ================= P5: L2 scan ================================
            with tc.tile_pool(name="w5", bufs=1) as wp, \
                 tc.tile_pool(name="st5", bufs=1) as stp, \
                 tc.tile_pool(name="p5", bufs=4) as sp5, \
                 tc.tile_pool(name="p5g", bufs=3, space="PSUM") as psg:
                w2x = {}
                w2h = {}
                for d in range(2):
                    for k in range(KI2):
                        w = wp.tile([128, 4 * H2], bf16, tag=f"w2x_{d}_{k}")
                        nc.sync.dma_start(w[:], w2x_d[d, 128 * k:128 * (k + 1), :])
                        w2x[(d, k)] = w
                    for k in range(KB2):
                        w = wp.tile([128, 4 * H2], bf16, tag=f"w2h_{d}_{k}")
                        nc.sync.dma_start(w[:], w2h_d[d, 128 * k:128 * (k + 1), :])
                        w2h[(d, k)] = w
                b2t = []
                for d in range(2):
                    b = wp.tile([NC2, 128], bf16, tag=f"b2_{d}")
                    nc.sync.dma_start(b[:], b2_d[d])
                    b2t.append(b)
                sel8 = wp.tile([NC2, 256], bf16, tag="sel8")
                nc.sync.dma_start(sel8[:], sel8_d[:])

                c2 = {}
                h0t2 = []
                for d in range(2):
                    for par in range(2):
                        c = stp.tile([128, 8 * KB2], f16, tag=f"c2_{d}_{par}")
                        c2[(d, par)] = c
                    nc.sync.dma_start(c2[(d, 0)][:], c0l2_d[d])
                    h0 = stp.tile([128, 8 * KB2], bf16, tag=f"h02_{d}")
                    nc.sync.dma_start(h0[:], h0l2_d[d])
                    h0t2.append(h0)

                for g in range(T // 4 if PH >= 5 else 0):
                    s0 = 4 * g
                    pgs5 = []
                    for d in range(2):
                        pg = psg.tile([128, 256], f32, space="PSUM", tag=f"pg5{d}")
                        pgs5.append(pg)
                        tb = s0 if d == 0 else T - 4 - s0
                        nc.tensor.matmul(pg[:], lhsT=b2t[d][:],
                                         rhs=sel8[:], start=True, stop=False,
                                         skip_group_check=True)
                        for c in range(NC2):
                            for k in range(KI2):
                                src = stash1[k // KB1][
                                    :, (k % KB1) * NTOK + 8 * tb:
                                    (k % KB1) * NTOK + 8 * tb + 32]
                                nc.tensor.matmul(
                                    pg[:, 32 * c:32 * c + 32],
                                    lhsT=w2x[(d, k)][:, 128 * c:128 * (c + 1)],
                                    rhs=src, start=False, stop=False,
                                    skip_group_check=True)
                    for i in range(4):
                        s = s0 + i
                        tps = [i, 3 - i]
                        ts = [s, T - 1 - s]
                        ves = [nc.vector, nc.gpsimd]
                        pgvs5 = [pgs5[d][:].rearrange("p (c q) -> p c q", q=32)
                                 for d in range(2)]

                        def hmm2(d, c):
                            tp = tps[d]
                            for k in range(KB2):
                                if s == 0:
                                    rhs = h0t2[d][:, 8 * k:8 * k + 8]
                                else:
                                    tprev = (s - 1) if d == 0 else (T - s)
                                    o = k * NTOK + 8 * tprev
                                    rhs = stash2[d][:, o:o + 8]
                                nc.tensor.matmul(
                                    pgs5[d][:, 32 * c + 8 * tp:
                                            32 * c + 8 * tp + 8],
                                    lhsT=w2h[(d, k)][:, 128 * c:128 * (c + 1)],
                                    rhs=rhs, start=False, stop=(k == KB2 - 1),
                                    skip_group_check=True)
                        for d in range(2):
                            for c in range(NC2):
                                hmm2(d, c)
                        sgs, sos, scs = [], [], []
                        for d in range(2):
                            sg = sp5.tile([128, 64], bf16, tag=f"sg5{d}")
                            nc.scalar.activation(
                                sg[:].rearrange("p (c q) -> p c q", q=8),
                                pgvs5[d][:, 0:8, 8 * tps[d]:8 * tps[d] + 8],
                                ACT.Sigmoid)
                            sgs.append(sg)
                            sos.append(sg[:, 48:64])
                        t1s = []
                        for d in range(2):
                            sg = sgs[d]
                            t1 = sp5.tile([128, 16], f16, tag=f"t15{d}")
                            nc.vector.tensor_tensor(out=t1[:], in0=sg[:, 32:48],
                                                    in1=c2[(d, s % 2)][:],
                                                    op=MULT)
                            t1s.append(t1)
                        for d in range(2):
                            sg = sgs[d]
                            gt = sp5.tile([128, 16], bf16, tag=f"gt5{d}")
                            nc.vector.tensor_scalar(gt[:], sg[:, 0:16], 2.0,
                                                    -1.0, MULT, ADD)
                            t2 = sp5.tile([128, 16], f16, tag=f"t25{d}")
                            nc.vector.tensor_tensor(out=t2[:], in0=sg[:, 16:32],
                                                    in1=gt[:], op=MULT)
                            nc.vector.tensor_tensor(out=c2[(d, (s + 1) % 2)][:],
                                                    in0=t1s[d][:], in1=t2[:],
                                                    op=ADD)
                        for d in range(2):
                            sc = sp5.tile([128, 16], bf16, tag=f"sc5{d}")
                            nc.scalar.activation(sc[:], c2[(d, (s + 1) % 2)][:],
                                                 ACT.Tanh)
                            scs.append(sc)
                        for d in range(2):
                            hout = stash2[d][:].rearrange(
                                "p (b n) -> p b n",
                                b=KB2)[:, :, 8 * ts[d]:8 * ts[d] + 8]
                            nc.vector.tensor_tensor(
                                out=hout,
                                in0=sos[d].rearrange("p (b q) -> p b q", q=8),
                                in1=scs[d][:].rearrange("p (b q) -> p b q", q=8),
                                op=MULT)

            # ================= P6: linear -> Esb + logits ==================
            epool_cm = tc.tile_pool(name="e67", bufs=1)
            epool = epool_cm.__enter__()
            Esb = epool.tile([TAGS, NTOK], bf16, tag="Esb")
            with tc.tile_pool(name="w6", bufs=1) as wp, \
                 tc.tile_pool(name="p6", bufs=3) as sp, \
                 tc.tile_pool(name="p6p", bufs=4, space="PSUM") as psp:
                lw = [wp.tile([128, TAGS], bf16, tag=f"lw{q}", name=f"lw{q}")
                      for q in range(4)]
                for q in range(4):
                    nc.sync.dma_start(lw[q][:], linWT[128 * q:128 * (q + 1), :])
                lb = wp.tile([1, TAGS], bf16, tag="lb")
                nc.sync.dma_start(lb[:], lin_b[:])
                # logitsT [48, NTOK] (+ exp) straight from the h2 stash
                for n in range(NTOK // 512 if PH >= 6 else 0):
                    pg = psp.tile([TAGS, 512], f32, space="PSUM", tag="pl")
                    for q in range(4):
                        d, bq = q // 2, q % 2
                        nc.tensor.matmul(
                            pg[:], lhsT=lw[q][:],
                            rhs=stash2[d][:, bq * NTOK + 512 * n:
                                          bq * NTOK + 512 * (n + 1)],
                            start=(q == 0), stop=False)
                    nc.tensor.matmul(pg[:], lhsT=lb[0:1, :], rhs=ones_bf[0:1, :],
                                     start=False, stop=True)
                    nc.scalar.activation(Esb[:, 512 * n:512 * (n + 1)],
                                         pg[:], ACT.Exp)
                # logits [NTOK, 48] to DRAM for gathers
                for m in range(NM if PH >= 6 else 0):
                    pg = psp.tile([128, TAGS], f32, space="PSUM", tag="pl2")
                    for q in range(4):
                        d, bq = q // 2, q % 2
                        nc.tensor.matmul(
                            pg[:], lhsT=stash2[d][:, bq * NTOK + 128 * m:
                                                  bq * NTOK + 128 * (m + 1)],
                            rhs=lw[q][:], start=(q == 0), stop=False)
                    nc.tensor.matmul(pg[:], lhsT=ones_bf[0:1, 0:128], rhs=lb[0:1, :],
                                     start=False, stop=True)
                    ls = sp.tile([128, TAGS], f32, tag="ls")
                    nc.scalar.copy(ls[:], pg[:])
                    nc.sync.dma_start(logits_t[128 * m:128 * (m + 1), :], ls[:])

            # ================= P8: gold path score =========================
            with tc.tile_pool(name="p8", bufs=3) as sp, \
                 tc.tile_pool(name="p8a", bufs=1) as aw, \
                 tc.tile_pool(name="p8p", bufs=2, space="PSUM") as psp:
                accW = aw.tile([128, NM], f32, tag="accW")
                lfl = logits_t[:].rearrange("n k -> (n k)").unsqueeze(1)
                tfl = trans_d[:].rearrange("a b -> (a b)").unsqueeze(1)
                for m in range(NM if PH >= 8 else 0):
                    ie = sp.tile([128, 1], i32, tag="ie")
                    nc.sync.dma_start(ie[:], idx_emit[128 * m:128 * (m + 1), :])
                    it = sp.tile([128, 1], i32, tag="it")
                    nc.sync.dma_start(it[:], idx_trans[128 * m:128 * (m + 1), :])
                    ge = sp.tile([128, 1], f32, tag="ge")
                    nc.gpsimd.indirect_dma_start(
                        out=ge[:], out_offset=None, in_=lfl,
                        in_offset=bass.IndirectOffsetOnAxis(ap=ie[:, :1], axis=0))
                    gt = sp.tile([128, 1], f32, tag="gt")
                    nc.gpsimd.indirect_dma_start(
                        out=gt[:], out_offset=None, in_=tfl,
                        in_offset=bass.IndirectOffsetOnAxis(ap=it[:, :1], axis=0))
                    if m == NM - 1:
                        nc.vector.tensor_tensor(out=gt[:], in0=gt[:],
                                                in1=mask_sb[:], op=MULT)
                    nc.vector.tensor_tensor(out=accW[:, m:m + 1], in0=ge[:],
                                            in1=gt[:], op=ADD)
                se16 = sp.tile([16, 1], f32, tag="se16")
                ist = sp.tile([BPC, 1], i32, tag="ist")
                nc.sync.dma_start(ist[:], idx_start[:])
                nc.gpsimd.indirect_dma_start(
                    out=se16[0:8, :], out_offset=None,
                    in_=start_d[:].rearrange("a b -> (a b)").unsqueeze(1),
                    in_offset=bass.IndirectOffsetOnAxis(ap=ist[:, :1], axis=0))
                ien = sp.tile([BPC, 1], i32, tag="ien")
                nc.sync.dma_start(ien[:], idx_end[:])
                nc.gpsimd.indirect_dma_start(
                    out=se16[8:16, :], out_offset=None,
                    in_=end_d[:].rearrange("a b -> (a b)").unsqueeze(1),
                    in_offset=bass.IndirectOffsetOnAxis(ap=ien[:, :1], axis=0))
                if PH < 8:
                    nc.gpsimd.memset(accW[:], 0.0)
                s1 = psp.tile([NM, BPC], f32, space="PSUM", tag="s1")
                nc.tensor.matmul(s1[:], lhsT=accW[:], rhs=sel_sb[:],
                                 start=True, stop=True)
                s1s = sp.tile([NM, BPC], f32, tag="s1s")
                nc.scalar.copy(s1s[:], s1[:])
                s2 = psp.tile([1, BPC], f32, space="PSUM", tag="s2")
                nc.tensor.matmul(s2[:], lhsT=ones_f[0:NM, 0:1], rhs=s1s[:],
                                 start=True, stop=False)
                nc.tensor.matmul(s2[:], lhsT=se16[:], rhs=sel_sb[0:16, :],
                                 start=False, stop=True)
                joint = sp.tile([1, BPC], f32, tag="joint")
                nc.scalar.copy(joint[:], s2[:])
                nc.sync.dma_start(out_d[0:1, :], joint[:])

            # ================= P7: CRF partition (exp domain) ==============
            with tc.tile_pool(name="p7", bufs=1) as sp, \
                 tc.tile_pool(name="p7a", bufs=4) as ap, \
                 tc.tile_pool(name="p7p", bufs=2, space="PSUM") as psp, \
                 tc.tile_pool(name="p7f", bufs=1, space="PSUM") as psf:
                tr = sp.tile([TAGS, TAGS], f32, tag="tr")
                nc.sync.dma_start(tr[:], trans_d[:])
                ETp = sp.tile([TAGS, TAGS], bf16, tag="ETp")
                nln48 = sp.tile([TAGS, 1], f32, tag="nln48")
                nc.gpsimd.memset(nln48[:], -LN48)
                nc.scalar.activation(ETp[:], tr[:], ACT.Exp, bias=nln48[:, 0:1])
                stv = sp.tile([TAGS, 1], f32, tag="stv")
                nc.sync.dma_start(stv[:], start_d[:])
                estart = sp.tile([TAGS, 1], f32, tag="estart")
                nc.scalar.activation(estart[:], stv[:], ACT.Exp)
                env = sp.tile([TAGS, 1], f32, tag="env")
                nc.sync.dma_start(env[:], end_d[:])
                eend = sp.tile([TAGS, 1], bf16, tag="eend")
                nc.scalar.activation(eend[:], env[:], ACT.Exp)

                alpha = ap.tile([TAGS, BPC], bf16, tag="alpha")
                if PH >= 7:
                    nc.vector.tensor_scalar_mul(alpha[:], Esb[:, 0:BPC],
                                                estart[:, 0:1])
                else:
                    nc.gpsimd.memset(alpha[:], 1.0)
                for t in range(1, T if PH >= 7 else 1):
                    pm = psp.tile([TAGS, BPC], f32, space="PSUM", tag="pm")
                    nc.tensor.matmul(pm[:], lhsT=ETp[:], rhs=alpha[:],
                                     start=True, stop=True)
                    alpha = ap.tile([TAGS, BPC], bf16, tag="alpha")
                    nc.vector.tensor_tensor(out=alpha[:], in0=pm[:],
                                            in1=Esb[:, BPC * t:BPC * (t + 1)], op=MULT)
                pf = psf.tile([1, BPC], f32, space="PSUM", tag="pf")
                nc.tensor.matmul(pf[:], lhsT=eend[:], rhs=alpha[:],
                                 start=True, stop=True)
                logz = sp.tile([1, BPC], f32, tag="logz")
                nc.scalar.activation(logz[:], pf[:], ACT.Ln)
                nc.vector.tensor_scalar_add(logz[:], logz[:], float((T - 1) * LN48))
                nc.sync.dma_start(out_d[1:2, :], logz[:])
            epool_cm.__exit__(None, None, None)
    _split_waits(nc, maxw=int(__import__("os").environ.get("KMAXW", "1")))
    return nc


def _split_waits(nc, maxw=2):
    """This container's walrus rejects instructions carrying more than a
    couple of semaphore waits. Hoist extras onto preceding same-engine
    NoOps (engines execute their stream in order, so this preserves the
    happens-before)."""
    import concourse.mybir as mybir
    import bass_rust
    compute_ops = {"Matmult", "Activation", "TensorTensor", "TensorScalar",
                   "TensorCopy", "TensorReduce", "Memset", "Iota",
                   "AffineSelect", "TensorTensorScan", "Select"}
    n_added = 0
    for fn in nc.m.functions:
        for blk in fn.blocks:
            insts = list(blk.instructions)
            out = []
            dirty = False
            for inst in insts:
                if str(inst.opcode) in compute_ops:
                    mw = 2 if maxw == 0 else max(1, maxw)
                else:
                    mw = 1  # DMAs/CTRL take at most one sync wait
                si = inst.sync_info
                if si is not None and len(si.on_wait) > mw:
                    waits = list(si.on_wait)
                    extra, keep = waits[:-mw], waits[-mw:]
                    for i in range(0, len(extra), mw):
                        nop = mybir.InstNoOp(
                            name=f"{inst.name}_hw{i}", ins=[], outs=[])
                        nop.engine = inst.engine
                        nop.sync_info = bass_rust.SyncInfo(
                            on_wait=extra[i:i + mw], on_update=[])
                        out.append(nop)
                        n_added += 1
                    inst.sync_info = bass_rust.SyncInfo(
                        on_wait=keep, on_update=list(si.on_update))
                    dirty = True
                out.append(inst)
            if dirty:
                blk.instructions = out
    return n_added


def _prep_inputs(inputs):
    import ml_dtypes
    bf = ml_dtypes.bfloat16

    g = {k: np.asarray(v) for k, v in inputs.items()}
    seq = g["sequences"].astype(np.int64)
    tags = g["tags"].astype(np.int64)

    # gate-chunk permutation: chunk c = 4*t_idx + b (L1) / 2*t_idx + b (L2),
    # t_idx order (g, i, f, o) from pytorch row order (i, f, g, o)
    def gate_perm(h, nb):
        return np.concatenate([
            np.arange(gs * h + 128 * b, gs * h + 128 * b + 128)
            for gs in [2, 0, 1, 3] for b in range(nb)])

    p1 = gate_perm(H1, KB1)
    p2 = gate_perm(H2, KB2)

    shared = {}
    shared["embed_bf"] = np.ascontiguousarray(g["embed_table"].astype(bf))
    for d, sfx in enumerate(["1f", "1b"]):
        wih = g["w_ih" + sfx][p1].astype(np.float64)  # [2048, 512] perm'd rows
        whh = g["w_hh" + sfx][p1].astype(np.float64)
        b = ((g["b_ih" + sfx] + g["b_hh" + sfx])[p1]).astype(np.float64)
        # tanh(x) = 2*sigmoid(2x)-1: pre-double the g-gate rows (chunks 0..3)
        wih[:4 * 128] *= 2.0
        whh[:4 * 128] *= 2.0
        b[:4 * 128] *= 2.0
        shared.setdefault("w1x", np.zeros((2, EMBED, 4 * H1), bf))[d] = wih.T.astype(bf)
        shared.setdefault("w1h", np.zeros((2, H1, 4 * H1), bf))[d] = whh.T.astype(bf)
        shared.setdefault("b1", np.zeros((2, NC1, 128), bf))[d] = \
            b.astype(bf).reshape(NC1, 128)
    for d, sfx in enumerate(["2f", "2b"]):
        wih = g["w_ih" + sfx][p2].astype(np.float64)
        whh = g["w_hh" + sfx][p2].astype(np.float64)
        b = ((g["b_ih" + sfx] + g["b_hh" + sfx])[p2]).astype(np.float64)
        wih[:2 * 128] *= 2.0
        whh[:2 * 128] *= 2.0
        b[:2 * 128] *= 2.0
        shared.setdefault("w2x", np.zeros((2, HID, 4 * H2), bf))[d] = wih.T.astype(bf)
        shared.setdefault("w2h", np.zeros((2, H2, 4 * H2), bf))[d] = whh.T.astype(bf)
        shared.setdefault("b2", np.zeros((2, NC2, 128), bf))[d] = \
            b.astype(bf).reshape(NC2, 128)
    shared["sel16"] = np.kron(np.eye(NC1), np.ones((1, 32))).astype(bf)
    shared["sel8"] = np.kron(np.eye(NC2), np.ones((1, 32))).astype(bf)
    shared["linWT"] = np.ascontiguousarray(g["lin_w"].T.astype(bf))
    shared["lin_b"] = g["lin_b"].astype(bf)[None, :]
    shared["transitions"] = g["transitions"].astype(np.float32)
    shared["start_trans"] = g["start_trans"].astype(np.float32)[:, None]
    shared["end_trans"] = g["end_trans"].astype(np.float32)[:, None]
    sel = (np.arange(128)[:, None] % 8 == np.arange(8)[None, :]).astype(np.float32)
    shared["sel"] = sel
    mask = np.ones((128, 1), np.float32)
    mask[120:128] = 0.0
    shared["mask_last"] = mask

    in_maps = []
    for cix in range(8):
        b0 = cix * BPC
        m = dict(shared)
        sl = slice(b0, b0 + BPC)
        seq_c = seq[sl]  # [8, T]
        tags_c = tags[sl]
        tok_seq = seq_c.T.reshape(NTOK).astype(np.int32)  # token-major (t, b)
        tok_tags = tags_c.T.reshape(NTOK).astype(np.int64)
        m["seq_tok"] = tok_seq[:, None]
        m["idx_emit"] = (np.arange(NTOK, dtype=np.int64) * TAGS + tok_tags).astype(np.int32)[:, None]
        nxt = np.concatenate([tok_tags[BPC:], np.zeros(BPC, np.int64)])
        it = tok_tags * TAGS + nxt
        it[-BPC:] = 0
        m["idx_trans"] = it.astype(np.int32)[:, None]
        m["idx_start"] = tags_c[:, 0].astype(np.int32)[:, None]
        m["idx_end"] = tags_c[:, T - 1].astype(np.int32)[:, None]

        # state layouts: [128p, (block, batch)]
        def pack_state(arr, nb):  # arr [8, nb*128] -> [2? no: [128, nb*8]
            return np.ascontiguousarray(
                arr.reshape(BPC, nb, 128).transpose(2, 1, 0).reshape(128, nb * BPC))

        m["h0l1"] = np.stack([pack_state(g["h0"][d, sl], KB1) for d in range(2)]).astype(bf)
        m["c0l1"] = np.stack([pack_state(g["c0"][d, sl], KB1) for d in range(2)]).astype(np.float16)
        m["h0l2"] = np.stack([pack_state(g["h1"][d, sl], KB2) for d in range(2)]).astype(bf)
        m["c0l2"] = np.stack([pack_state(g["c1"][d, sl], KB2) for d in range(2)]).astype(np.float16)
        in_maps.append(m)
    return in_maps


def kernel(**inputs) -> np.ndarray:
    import time
    from concourse.bass_utils import run_bass_kernel_spmd

    if "nc" not in _CACHE:
        _CACHE["nc"] = _build_program()
    nc = _CACHE["nc"]

    in_maps = _prep_inputs(inputs)
    res = None
    for attempt in range(3):
        try:
            res = run_bass_kernel_spmd(nc, in_maps, core_ids=list(range(8)))
            break
        except Exception:
            # transient NRT_EXEC_UNIT_UNRECOVERABLE after wedged runs —
            # observed to recover after ~60s
            if attempt == 2:
                raise
            time.sleep(60)
    loss = np.float64(0.0)
    for r in res.results:
        o = r["out"].astype(np.float64)  # [2, 8]: joint, logz
        loss += np.sum(o[0] - o[1])
    return np.float32(-loss)
